# revision 1
# baseline (speedup 1.0000x reference)
"""DeformableAttention1D on 8 TRN2 NeuronCores via Bass/Tile.

Sharding: core c handles offset-group g=c//2 (64 of 256 channels, 2 of 8 heads)
and query-half qh=c%2 (512 of 1024 positions). Each core computes its group's
offsets/gather/CPB/attention independently; the final output projection is
computed as a partial (wo sliced by group) and summed on the host (the
"all-reduce" of the output projection).

Device-side numerics: fp32 everywhere except the CPB relative-position-bias
MLP and its broadcast, which use fp32r matmuls (1 cycle/column vs 4 for fp32).
The ACT engine is restricted to ONE table set (natural_log_exp_and_others:
Exp/Ln/Relu/Copy/Identity/Square) because runtime table swaps are broken in
this environment; tanh and erf(gelu) are composed from Exp + DVE ops.
"""
import os
import sys

sys.path.insert(0, "/opt/trn_rl_repo")

DEBUG = bool(os.environ.get("DEFORM_DEBUG"))

import numpy as np

import concourse.bacc as bacc
import concourse.bass as bass
import concourse.mybir as mybir
import concourse.tile as tile
import concourse.bass_utils as bass_utils

F32 = mybir.dt.float32
F32R = mybir.dt.float32r
I32 = mybir.dt.int32
U32 = mybir.dt.uint32
AF = mybir.ActivationFunctionType
ALU = mybir.AluOpType

# model dims (hardcoded per problem spec)
DIM = 256
N = 1024
G = 4
HEADS = 8
DH = 32
NDS = 256          # downsampled kv positions
QS = 512           # queries per core
DPG = 64           # channels per group
OFF_K = 6
DS = 4             # downsample stride
OFF_SCALE = 4.0
NCORES = 8

# A&S 7.1.26 erf coefficients (|err| <= 1.5e-7)
ERF_P = 0.3275911
ERF_A = [0.254829592, -0.284496736, 1.421413741, -1.453152027, 1.061405429]

_CACHED = {}


def _patch_act_tables():
    """Restrict activation-table selection to the single set that covers all
    ACT functions used by this kernel, so exactly one table load is emitted
    (runtime table swaps do not work in this environment)."""
    import concourse.hw_specs as hw_specs

    if getattr(bacc, "_deform_act_patch", False):
        return
    orig = hw_specs.get_activation_tables

    keep = "natural_log_exp_and_others"

    def patched(module_arch):
        tabs = orig(module_arch)
        keep_funcs = tabs[keep]
        out = {}
        for name, funcs in tabs.items():
            if name == keep:
                out[name] = funcs
            else:
                out[name] = funcs - keep_funcs
        return out

    bacc.get_activation_tables = patched
    bacc._deform_act_patch = True


def _erf_gelu(nc, sb, out_ap, x_ap, shape):
    """out = 0.5 * x * (1 + erf(x/sqrt(2))) via A&S 7.1.26 (no erf table).

    Writes (1 + erf(x/sqrt2)) * x  (WITHOUT the 0.5 -- folded into wproj).
    """
    P, Nf = shape
    sq = sb.tile([P, Nf], F32, name="gelu_sq", tag="gelu_sq")
    nc.scalar.activation(sq[:], x_ap, AF.Square)
    e = sb.tile([P, Nf], F32, name="gelu_e", tag="gelu_e")
    # e = exp(-x^2/2)
    nc.scalar.activation(e[:], sq[:], AF.Exp, scale=-0.5)
    ax = sb.tile([P, Nf], F32, name="gelu_ax", tag="gelu_ax")
    # |x|/sqrt(2) = max(x, -x) * (1/sqrt2): two steps
    nc.vector.scalar_tensor_tensor(ax[:], x_ap, -1.0, x_ap, ALU.mult, ALU.max)
    t = sb.tile([P, Nf], F32, name="gelu_t", tag="gelu_t")
    # t = 1 / (1 + p * |x| / sqrt2)
    nc.vector.tensor_scalar(t[:], ax[:], float(ERF_P / np.sqrt(2.0)), 1.0, ALU.mult, ALU.add)
    nc.vector.reciprocal(t[:], t[:])
    poly = sb.tile([P, Nf], F32, name="gelu_poly", tag="gelu_poly")
    # P(t) = a1 t + a2 t^2 + ... + a5 t^5 via (x + c)*t nested form
    nc.vector.tensor_scalar(poly[:], t[:], ERF_A[4], ERF_A[3], ALU.mult, ALU.add)
    nc.vector.tensor_tensor(poly[:], poly[:], t[:], ALU.mult)
    nc.vector.scalar_tensor_tensor(poly[:], poly[:], ERF_A[2], t[:], ALU.add, ALU.mult)
    nc.vector.scalar_tensor_tensor(poly[:], poly[:], ERF_A[1], t[:], ALU.add, ALU.mult)
    nc.vector.scalar_tensor_tensor(poly[:], poly[:], ERF_A[0], t[:], ALU.add, ALU.mult)
    # poly*e = 1 - erf(|x|/sqrt2)  =>  erfa = 1 - poly*e
    erfa = sb.tile([P, Nf], F32, name="gelu_erfa", tag="gelu_erfa")
    nc.vector.tensor_tensor(erfa[:], poly[:], e[:], ALU.mult)
    nc.vector.tensor_scalar(erfa[:], erfa[:], -1.0, 1.0, ALU.mult, ALU.add)
    # copysign: erf(x) = sign(x)*erfa
    sgn = sb.tile([P, Nf], U32, name="gelu_sgn", tag="gelu_sgn")
    nc.vector.tensor_scalar(sgn[:], x_ap.bitcast(U32), 0x80000000, None, ALU.bitwise_and)
    erfs = sb.tile([P, Nf], F32, name="gelu_erfs", tag="gelu_erfs")
    nc.vector.tensor_tensor(erfs[:].bitcast(U32), erfa[:].bitcast(U32), sgn[:], ALU.bitwise_or)
    # out = (1 + erf) * x    (0.5 folded into wproj)
    nc.vector.tensor_scalar(erfs[:], erfs[:], 1.0, None, ALU.add)
    nc.vector.tensor_tensor(out_ap, erfs[:], x_ap, ALU.mult)


def _tanh_rows(nc, sb, out_ap, x_ap, shape):
    """out = tanh(x) = sign(x) * (1 - 2/(exp(2*min(|x|,30))+1)) on small tiles."""
    P, Nf = shape
    ax = sb.tile([P, Nf], F32, name="th_ax", tag="th_ax")
    nc.vector.scalar_tensor_tensor(ax[:], x_ap, -1.0, x_ap, ALU.mult, ALU.max)
    nc.vector.tensor_scalar(ax[:], ax[:], 30.0, None, ALU.min)
    e = sb.tile([P, Nf], F32, name="th_e", tag="th_e")
    nc.scalar.activation(e[:], ax[:], AF.Exp, scale=2.0)
    nc.vector.tensor_scalar(e[:], e[:], 1.0, None, ALU.add)
    r = sb.tile([P, Nf], F32, name="th_r", tag="th_r")
    nc.vector.reciprocal(r[:], e[:])
    # tha = 1 - 2r
    nc.vector.tensor_scalar(r[:], r[:], -2.0, 1.0, ALU.mult, ALU.add)
    sgn = sb.tile([P, Nf], U32, name="th_sgn", tag="th_sgn")
    nc.vector.tensor_scalar(sgn[:], x_ap.bitcast(U32), 0x80000000, None, ALU.bitwise_and)
    nc.vector.tensor_tensor(out_ap.bitcast(U32), r[:].bitcast(U32), sgn[:], ALU.bitwise_or)


def build_nc():
    _patch_act_tables()
    nc = bacc.Bacc("TRN2", target_bir_lowering=False, debug=False, num_devices=NCORES)

    # ---- per-core DRAM inputs ----
    din = {}

    def dt_in(name, shape):
        din[name] = nc.dram_tensor(name, shape, F32, kind="ExternalInput")
        return din[name]

    dt_in("xg", [DPG, N])
    dt_in("xq", [DPG, QS])
    dt_in("mask_st", [128, 32 * 128])
    # all small weights + identity packed into one tensor (one DMA)
    dt_in("packed", [128, 788])
    y_out = nc.dram_tensor("y", [DIM, QS], F32, kind="ExternalOutput")
    dbg = {}
    if DEBUG:
        for nm, shp in [("dbg_q", [DPG, N]), ("dbg_vgsp1", [1, NDS]),
                        ("dbg_rows4", [1, 4 * NDS]), ("dbg_kv", [DPG, NDS]),
                        ("dbg_k", [DPG, NDS]), ("dbg_v", [DPG, NDS]),
                        ("dbg_qs", [DPG, QS]), ("dbg_t0", [128, QS]),
                        ("dbg_bstk0", [128, NDS]), ("dbg_logit00", [128, QS]),
                        ("dbg_avn", [DPG, QS])]:
            dbg[nm] = nc.dram_tensor(nm, shp, F32, kind="ExternalOutput")

    NT = N // 128          # 8 n-tiles for gather
    NITER = QS // 2        # 256 CPB iterations (2 queries each)
    NSTACK = NITER // 32   # 8 bias stacks

    with tile.TileContext(nc) as tc:
        with (
            tc.tile_pool(name="const", bufs=1) as cst,
            tc.tile_pool(name="work", bufs=2) as wk,
            tc.tile_pool(name="rows", bufs=1) as rw,
            tc.tile_pool(name="persist", bufs=1) as pe_pool,
            tc.tile_pool(name="h1p", bufs=4) as h1p,
            tc.tile_pool(name="h2p", bufs=4) as h2p,
        ):
            # ---- load inputs: xg, xq, then one packed-weights DMA ----
            xg = cst.tile([DPG, N], F32, name="xg", tag="xg")
            nc.sync.dma_start(xg[:], din["xg"].ap())
            xq = cst.tile([DPG, QS], F32, name="xq", tag="xq")
            nc.sync.dma_start(xq[:], din["xq"].ap())
            packed = cst.tile([128, 788], F32, name="packed", tag="packed")
            nc.sync.dma_start(packed[:], din["packed"].ap())
            w2bd = packed[:, 0:128]
            eyet = packed[:, 128:256]
            wqT = packed[0:DPG, 256:320]
            wqTs = packed[0:DPG, 320:384]
            wkT = packed[0:DPG, 384:448]
            wvT = packed[0:DPG, 448:512]
            woT = packed[0:DPG, 512:768]
            wdw = packed[0:DPG, 768:774]
            bodw = packed[0:DPG, 774:775]
            wproj_half = packed[0:DPG, 775:776]
            b1col = packed[:, 776:777]
            b2col = packed[:, 777:778]
            b3bc = packed[:, 778:780]
            qbase = packed[:, 780:781]
            w3bd = packed[:, 781:785]

            ones_col = cst.tile([128, 1], F32, name="ones", tag="ones")
            nc.gpsimd.memset(ones_col[:], 1.0)
            # dummy activation: triggers the (single) ACT table load at t=0 so
            # it overlaps the input DMAs instead of sitting in the offsets chain
            warm = cst.tile([128, 1], F32, name="warm", tag="warm")
            nc.scalar.activation(warm[:], ones_col[:], AF.Relu)
            ones_colr = cst.tile([128, 1], F32R, name="onesr", tag="onesr")
            nc.vector.tensor_copy(ones_colr[:], ones_col[:])

            # fp32r copies of CPB weights
            w2bdr = cst.tile([128, 128], F32R, name="w2bdr", tag="w2bdr")
            nc.vector.tensor_copy(w2bdr[:], w2bd)
            w3bdr = cst.tile([128, 4], F32R, name="w3bdr", tag="w3bdr")
            nc.vector.tensor_copy(w3bdr[:], w3bd)
            woTr = cst.tile([DPG, DIM], F32R, name="woTr", tag="woTr")
            nc.vector.tensor_copy(woTr[:], woT)


            # persistent SBUF tiles that cross phase boundaries
            k_sb = pe_pool.tile([DPG, NDS], F32R, name="k_sb", tag="k_sb")
            qs_sb = pe_pool.tile([DPG, QS], F32R, name="qs_sb", tag="qs_sb")
            vT = [pe_pool.tile([128, DPG], F32R, name=f"vT{H}", tag=f"vT{H}") for H in range(2)]
            tT = [pe_pool.tile([128, NDS], F32R, name=f"tT{t}", tag=f"tT{t}") for t in range(4)]
            # bias, transposed into attention layout, packed as
            # col = ((((itq*2 + itl)*32 + pp)*2 + h1)*2 + c)*2 + o  (j on partitions)
            biasT_sb = pe_pool.tile([128, 4 * QS], F32, name="biasT_sb", tag="biasT_sb")
            avn = pe_pool.tile([DPG, QS], F32R, name="avn", tag="avn")

            # ============ phases A-C: q, offsets, gather, kv, t ============
            with tc.tile_pool(name="psA", bufs=2, space="PSUM") as psA:
                # ---- phase A: q + offsets ----
                q_pad = pe_pool.tile([DPG, N + 2], F32, name="q_pad", tag="q_pad")
                nc.gpsimd.memset(q_pad[:], 0.0)
                for h in range(2):
                    pq = psA.tile([DPG, QS], F32, name="pA512", tag="pA512")
                    nc.tensor.matmul(pq[:], wqT, xg[:, h * QS:(h + 1) * QS])
                    nc.scalar.copy(q_pad[:, 1 + h * QS:1 + (h + 1) * QS], pq[:])

                # depthwise strided conv (6 taps)
                acc = wk.tile([DPG, NDS], F32, name="conv_acc", tag="conv_acc")
                nc.vector.tensor_scalar(
                    acc[:], q_pad[:, 0:N - 3:DS], wdw[:, 0:1], bodw, ALU.mult, ALU.add)
                for kk in range(1, OFF_K):
                    nc.vector.scalar_tensor_tensor(
                        acc[:], q_pad[:, kk:kk + N - 3:DS], wdw[:, kk:kk + 1], acc[:],
                        ALU.mult, ALU.add)

                if DEBUG:
                    nc.sync.dma_start(dbg["dbg_q"].ap(), q_pad[:, 1:N + 1])
                gl = wk.tile([DPG, NDS], F32, name="gelu_out", tag="gelu_out")
                _erf_gelu(nc, wk, gl[:], acc[:], [DPG, NDS])

                # proj row: [1, NDS] = sum_c 0.5*wproj[c] * gl[c, :]
                pproj = psA.tile([1, NDS], F32, name="pproj", tag="pproj")
                nc.tensor.matmul(pproj[:], wproj_half, gl[:])
                proj_sb = rw.tile([1, NDS], F32, name="proj_sb", tag="proj_sb")
                nc.vector.tensor_copy(proj_sb[:], pproj[:])
                th = rw.tile([1, NDS], F32, name="th", tag="th")
                _tanh_rows(nc, rw, th[:], proj_sb[:], [1, NDS])

                # vgrid = j + 4*tanh ; vgsp1 = vgrid*2/255 ; p_pix = vgsp1*512 - 0.5
                iotaj = rw.tile([1, NDS], I32, name="iotaj", tag="iotaj")
                nc.gpsimd.iota(iotaj[:], pattern=[[1, NDS]], base=0, channel_multiplier=0)
                iotajf = rw.tile([1, NDS], F32, name="iotajf", tag="iotajf")
                nc.vector.tensor_copy(iotajf[:], iotaj[:])
                vgrid = rw.tile([1, NDS], F32, name="vgrid", tag="vgrid")
                nc.vector.scalar_tensor_tensor(vgrid[:], th[:], OFF_SCALE, iotajf[:], ALU.mult, ALU.add)
                vgsp1 = rw.tile([1, NDS], F32, name="vgsp1", tag="vgsp1")
                nc.vector.tensor_scalar(vgsp1[:], vgrid[:], float(2.0 / (NDS - 1)), None, ALU.mult)
                ppix = rw.tile([1, NDS], F32, name="ppix", tag="ppix")
                nc.vector.tensor_scalar(ppix[:], vgsp1[:], float(N / 2.0), -0.5, ALU.mult, ALU.add)

                # rows4 = [i0f | i1f | w0 | w1]
                rows4 = rw.tile([1, 4 * NDS], F32, name="rows4", tag="rows4")
                i0i = rw.tile([1, NDS], I32, name="i0i", tag="i0i")
                nc.vector.tensor_copy(i0i[:], ppix[:])
                i0c = rw.tile([1, NDS], F32, name="i0c", tag="i0c")
                nc.vector.tensor_copy(i0c[:], i0i[:])
                # floor(p) regardless of the convert rounding mode:
                # i0 = cvt(p) - (cvt(p) > p)
                gt = rw.tile([1, NDS], F32, name="gt", tag="gt")
                nc.vector.tensor_tensor(gt[:], i0c[:], ppix[:], ALU.is_gt)
                nc.vector.tensor_tensor(rows4[:, 0:NDS], i0c[:], gt[:], ALU.subtract)
                nc.vector.tensor_scalar(rows4[:, NDS:2 * NDS], rows4[:, 0:NDS], 1.0, None, ALU.add)
                nc.vector.tensor_tensor(rows4[:, 3 * NDS:4 * NDS], ppix[:], rows4[:, 0:NDS], ALU.subtract)
                nc.vector.tensor_scalar(rows4[:, 2 * NDS:3 * NDS], rows4[:, 3 * NDS:4 * NDS], -1.0, 1.0, ALU.mult, ALU.add)

                if DEBUG:
                    nc.sync.dma_start(dbg["dbg_vgsp1"].ap(), vgsp1[:])
                    nc.sync.dma_start(dbg["dbg_rows4"].ap(), rows4[:])
                bc4 = pe_pool.tile([128, 4 * NDS], F32, name="bc4", tag="bc4")
                nc.gpsimd.partition_broadcast(bc4[:], rows4[:])

                # vgsp1 as per-partition columns for the two j-halves
                # (PE transpose of the row -- avoids DMA queue latency)
                vgsp1c = cst.tile([128, 2], F32, name="vgsp1c", tag="vgsp1c")
                for H in range(2):
                    ptv = psA.tile([128, 128], F32, name="ptv", tag="ptp")
                    nc.tensor.transpose(ptv[:, 0:1], vgsp1[:, H * 128:(H + 1) * 128],
                                        eyet[0:1, 0:1])
                    nc.vector.tensor_copy(vgsp1c[:, H:H + 1], ptv[:, 0:1])

                # CPB selection masks (one prepacked DMA + fp32r round)
                maskr = pe_pool.tile([128, 32 * 128], F32R, name="maskr", tag="maskr")
                with tc.tile_pool(name="maskst", bufs=1) as mp:
                    mask_st = mp.tile([128, 32 * 128], F32, name="mask_st", tag="mask_st")
                    nc.sync.dma_start(mask_st[:], din["mask_st"].ap())
                    nc.vector.tensor_copy(maskr[:], mask_st[:])

                # ---- phase C: t = sign(pos)*log1p(|pos|), transposed ----
                io = wk.tile([128, QS], I32, name="io", tag="io")
                nc.gpsimd.iota(io[:], pattern=[[1, QS]], base=0, channel_multiplier=0)
                gqp = wk.tile([128, QS], F32, name="gqp", tag="gqp")
                nc.vector.tensor_scalar(gqp[:], io[:], qbase, float(2.0 / (N - 1)), ALU.add, ALU.mult)

                for H in range(2):
                    pos = wk.tile([128, QS], F32, name="pos", tag="pos")
                    nc.vector.tensor_scalar(pos[:], gqp[:], vgsp1c[:, H:H + 1], None, ALU.subtract)
                    apos = wk.tile([128, QS], F32, name="apos", tag="apos")
                    nc.vector.scalar_tensor_tensor(apos[:], pos[:], -1.0, pos[:], ALU.mult, ALU.max)
                    aln = wk.tile([128, QS], F32, name="aln", tag="aln")
                    nc.scalar.activation(aln[:], apos[:], AF.Ln, bias=1.0)
                    sgn = wk.tile([128, QS], U32, name="psgn", tag="psgn")
                    nc.vector.tensor_scalar(sgn[:], pos[:].bitcast(U32), 0x80000000, None, ALU.bitwise_and)
                    t_H = wk.tile([128, QS], F32, name="t_H", tag="t_H")
                    nc.vector.tensor_tensor(t_H[:].bitcast(U32), aln[:].bitcast(U32), sgn[:], ALU.bitwise_or)
                    if DEBUG and H == 0:
                        nc.sync.dma_start(dbg["dbg_t0"].ap(), t_H[:])
                    for it in range(4):
                        ptp = psA.tile([128, 128], F32, name="ptp", tag="ptp")
                        nc.tensor.transpose(ptp[:], t_H[:, it * 128:(it + 1) * 128], eyet)
                        nc.scalar.copy(tT[it][:, H * 128:(H + 1) * 128], ptp[:])

                # selection masks for the CPB broadcast. Iteration p reads
                # tT rows (2p, 2p+1); those sit inside the 32-aligned window
                # [32*(p//16), +32), so a [32, 128] mask indexed by p%16
                # suffices (16 variants).

                # ---- phase B: gather (one-hot matmul), kv, k, v, vT ----
                xgT = []
                for t in range(NT):
                    pt = psA.tile([128, 128], F32, name="ptp", tag="ptp")
                    nc.tensor.transpose(pt[:, 0:DPG], xg[:, t * 128:(t + 1) * 128], eyet[0:DPG, 0:DPG])
                    st = pe_pool.tile([128, DPG], F32, name=f"xgT{t}", tag=f"xgT{t}")
                    nc.scalar.copy(st[:], pt[:, 0:DPG])
                    xgT.append(st)

                pkv = psA.tile([DPG, NDS], F32, name="pA256", tag="pA256")
                for t in range(NT):
                    icol = wk.tile([128, 1], I32, name="icol", tag="icol")
                    nc.gpsimd.iota(icol[:], pattern=[[0, 1]], base=t * 128, channel_multiplier=1)
                    icolf = wk.tile([128, 1], F32, name="icolf", tag="icolf")
                    nc.vector.tensor_copy(icolf[:], icol[:])
                    eq0 = wk.tile([128, NDS], F32, name="eq0", tag="eq0")
                    nc.vector.tensor_scalar(eq0[:], bc4[:, 0:NDS], icolf[:], None, ALU.is_equal)
                    s0 = wk.tile([128, NDS], F32, name="s0", tag="s0")
                    nc.vector.tensor_tensor(s0[:], eq0[:], bc4[:, 2 * NDS:3 * NDS], ALU.mult)
                    eq1 = wk.tile([128, NDS], F32, name="eq1", tag="eq1")
                    nc.vector.tensor_scalar(eq1[:], bc4[:, NDS:2 * NDS], icolf[:], None, ALU.is_equal)
                    s1 = wk.tile([128, NDS], F32, name="s1", tag="s1")
                    nc.vector.tensor_tensor(s1[:], eq1[:], bc4[:, 3 * NDS:4 * NDS], ALU.mult)
                    S = wk.tile([128, NDS], F32, name="S", tag="S")
                    nc.vector.tensor_tensor(S[:], s0[:], s1[:], ALU.add)
                    nc.tensor.matmul(pkv[:], xgT[t][:], S[:],
                                     start=(t == 0), stop=(t == NT - 1))
                kv = wk.tile([DPG, NDS], F32, name="kv", tag="kv")
                nc.scalar.copy(kv[:], pkv[:])
                if DEBUG:
                    nc.sync.dma_start(dbg["dbg_kv"].ap(), kv[:])

                pk = psA.tile([DPG, NDS], F32, name="pA256", tag="pA256")
                nc.tensor.matmul(pk[:], wkT, kv[:])
                nc.scalar.copy(k_sb[:], pk[:])
                pv = psA.tile([DPG, NDS], F32, name="pA256", tag="pA256")
                nc.tensor.matmul(pv[:], wvT, kv[:])
                v_sb = wk.tile([DPG, NDS], F32, name="v_sb", tag="v_sb")
                nc.scalar.copy(v_sb[:], pv[:])
                if DEBUG:
                    nc.sync.dma_start(dbg["dbg_k"].ap(), k_sb[:])
                    nc.sync.dma_start(dbg["dbg_v"].ap(), v_sb[:])

                for H in range(2):
                    pt = psA.tile([128, 128], F32, name="ptp", tag="ptp")
                    nc.tensor.transpose(pt[:, 0:DPG], v_sb[:, H * 128:(H + 1) * 128], eyet[0:DPG, 0:DPG])
                    nc.scalar.copy(vT[H][:], pt[:, 0:DPG])

                # q_s (scaled q for this core's query half)
                pqs = psA.tile([DPG, QS], F32, name="pA512", tag="pA512")
                nc.tensor.matmul(pqs[:], wqTs, xq[:])
                nc.scalar.copy(qs_sb[:], pqs[:])
                if DEBUG:
                    nc.sync.dma_start(dbg["dbg_qs"].ap(), qs_sb[:])


            # ============ phase D: CPB MLP (fp32r) ============
            with (
                tc.tile_pool(name="ps1", bufs=4, space="PSUM") as ps1,
                tc.tile_pool(name="ps2", bufs=2, space="PSUM") as ps2,
                tc.tile_pool(name="ps3", bufs=1, space="PSUM") as ps3,
            ):
                # two psum banks collect the transposed bias (one per j-half),
                # spilled to SBUF when full (after every 64 pairs)
                biasTp = [ps3.tile([128, 2 * NDS], F32, name=f"biasTp{i}", tag=f"biasTp{i}")
                          for i in range(2)]
                for it in range(4):
                    for pp in range(32):  # two queries... pair-iteration
                        kpair = it * 32 + pp
                        pre1 = ps1.tile([128, 2 * NDS], F32, name="pre1", tag="pre1")
                        h1 = h1p.tile([128, 2 * NDS], F32R, name="h1", tag="h1")
                        pre2 = ps2.tile([128, 2 * NDS], F32, name="pre2", tag="pre2")
                        h2 = h2p.tile([128, 2 * NDS], F32R, name="h2", tag="h2")
                        for half in range(2):
                            p = 2 * pp + half
                            sl = slice(half * NDS, (half + 1) * NDS)
                            a, m = p // 32, p % 32
                            nc.tensor.matmul(
                                pre1[:, sl],
                                maskr[64 * a:64 * (a + 1), 128 * m:128 * (m + 1)],
                                tT[it][64 * a:64 * (a + 1), :])
                        r1_act = kpair % 2 == 1
                        r2_act = kpair % 2 == 0
                        if r1_act:
                            nc.scalar.activation(h1[:], pre1[:], AF.Relu, bias=b1col)
                        else:
                            nc.vector.tensor_scalar(h1[:], pre1[:], b1col, 0.0, ALU.add, ALU.max)
                        for half in range(2):
                            sl = slice(half * NDS, (half + 1) * NDS)
                            nc.tensor.matmul(pre2[:, sl], w2bdr[:], h1[:, sl])
                        if r2_act:
                            nc.scalar.activation(h2[:], pre2[:], AF.Relu, bias=b2col)
                        else:
                            nc.vector.tensor_scalar(h2[:], pre2[:], b2col, 0.0, ALU.add, ALU.max)
                        for h1f in range(2):
                            for H in range(2):
                                outn = 2 * kpair + h1f
                                slot = outn % 128
                                nc.tensor.matmul(
                                    biasTp[H][:, 4 * slot:4 * slot + 4],
                                    h2[:, 256 * h1f + 128 * H:256 * h1f + 128 * H + 128],
                                    w3bdr[:])
                        if kpair % 64 == 63:
                            bank = kpair // 64
                            nc.vector.tensor_copy(
                                biasT_sb[:, QS * bank:QS * (bank + 1)],
                                biasTp[0][:])
                            nc.scalar.copy(
                                biasT_sb[:, QS * (2 + bank):QS * (2 + bank + 1)],
                                biasTp[1][:])
                            if bank == 0:
                                biasTp = [ps3.tile([128, 2 * NDS], F32,
                                                   name=f"biasTp{i}b", tag=f"biasTp{i}")
                                          for i in range(2)]

            if DEBUG:
                nc.sync.dma_start(dbg["dbg_bstk0"].ap(), biasT_sb[:, 0:NDS])
            # ============ phase E: attention ============
            with (
                tc.tile_pool(name="psE", bufs=2, space="PSUM") as psE,
                tc.tile_pool(name="psE1", bufs=1, space="PSUM") as psE1,
            ):
                # biasT_sb column decomposition:
                # col = 2048*H? no: region (2H+bank)*QS, inner 4*slot + 2c + o
                # with slot = (2*(32it+pp) + h1) % 128 and bank = itq = it//2.
                # As i_loc = 128it + 4pp + 2h1 + c runs over [128it, 128it+128),
                # (pp, h1, c) iterate with c innermost -- matching free order.
                bview = biasT_sb[:].rearrange(
                    "p (r itl pp h1 c o) -> p r itl pp h1 c o",
                    r=4, itl=2, pp=32, h1=2, c=2, o=2)

                for h in range(2):
                    expT = []
                    for H in range(2):
                        psim = psE.tile([128, QS], F32, name="psim", tag="psim")
                        nc.tensor.matmul(
                            psim[:], k_sb[32 * h:32 * (h + 1), H * 128:(H + 1) * 128],
                            qs_sb[32 * h:32 * (h + 1), :])
                        logit = wk.tile([128, QS], F32, name="logit", tag="logit")
                        for it in range(4):
                            itq, itl = it // 2, it % 2
                            nc.vector.scalar_tensor_tensor(
                                logit[:, 128 * it:128 * (it + 1)],
                                bview[:, 2 * H + itq, itl, :, :, :, h],
                                b3bc[:, h:h + 1],
                                psim[:, 128 * it:128 * (it + 1)],
                                ALU.add, ALU.add)
                        if DEBUG and h == 0 and H == 0:
                            nc.sync.dma_start(dbg["dbg_logit00"].ap(), logit[:])
                        et = wk.tile([128, QS], F32R, name="expT", tag="expT")
                        nc.scalar.activation(et[:], logit[:], AF.Exp)
                        expT.append(et)

                    # sums over j via ones-matmul, then reciprocal
                    psum_s = psE1.tile([1, QS], F32, name="psum_s", tag="psum_s")
                    for H in range(2):
                        nc.tensor.matmul(psum_s[:], ones_colr[:], expT[H][:],
                                         start=(H == 0), stop=(H == 1))
                    rs = rw.tile([1, QS], F32, name="rs", tag="rs")
                    nc.vector.reciprocal(rs[:], psum_s[:])
                    rsb = wk.tile([32, QS], F32, name="rsb", tag="rsb")
                    nc.gpsimd.partition_broadcast(rsb[:], rs[:])

                    pav = psE1.tile([32, QS], F32, name="pav", tag="pav")
                    for H in range(2):
                        nc.tensor.matmul(pav[:], vT[H][:, 32 * h:32 * (h + 1)], expT[H][:],
                                         start=(H == 0), stop=(H == 1))
                    nc.vector.tensor_tensor(avn[32 * h:32 * (h + 1), :], pav[:], rsb[:], ALU.mult)

                for m in range(2):
                    py = psE.tile([128, QS], F32, name="py", tag="py")
                    nc.tensor.matmul(py[:], woTr[:, m * 128:(m + 1) * 128], avn[:])
                    y_sb = wk.tile([128, QS], F32, name="y_sb", tag="y_sb")
                    nc.scalar.copy(y_sb[:], py[:])
                    nc.sync.dma_start(y_out.ap()[m * 128:(m + 1) * 128, :], y_sb[:])

    nc.compile()
    return nc


def _shard_inputs(inputs):
    """Build the 8 per-core input maps from the full inputs."""
    x = np.ascontiguousarray(inputs["x"][0])              # [256, 1024]
    wq, wk, wv = inputs["wq"], inputs["wk"], inputs["wv"]  # [4, 64, 64]
    wo = inputs["wo"]                                      # [256, 256]
    w_off_dw = inputs["w_off_dw"][:, 0, :]                 # [64, 6]
    b_off_dw = inputs["b_off_dw"]                          # [64]
    w_off_proj = inputs["w_off_proj"]                      # [64]
    w1 = inputs["cpb_w1"][:, 0]                            # [64]
    b1 = inputs["cpb_b1"]                                  # [64]
    w2 = inputs["cpb_w2"]                                  # [64, 64]
    b2 = inputs["cpb_b2"]                                  # [64]
    w3 = inputs["cpb_w3"]                                  # [2, 64]
    b3 = inputs["cpb_b3"]                                  # [2]

    f = np.float32
    w1sel = np.zeros((2, 128), f)
    w1sel[0, :64] = w1
    w1sel[1, 64:] = w1
    mask_st = np.zeros((128, 32 * 128), f)
    for band in range(2):
        for m in range(32):
            mask_st[64 * band + 2 * m:64 * band + 2 * m + 2, 128 * m:128 * (m + 1)] = w1sel
    b1col = np.concatenate([b1, b1]).astype(f)[:, None]
    w2bd = np.zeros((128, 128), f)
    w2bd[:64, :64] = w2.T
    w2bd[64:, 64:] = w2.T
    b2col = np.concatenate([b2, b2]).astype(f)[:, None]
    w3bd = np.zeros((128, 4), f)
    w3bd[:64, :2] = w3.T
    w3bd[64:, 2:] = w3.T
    b3bc = np.broadcast_to(b3.astype(f)[None, :], (128, 2)).copy()
    base_packed = np.zeros((128, 788), f)
    base_packed[:, 0:128] = w2bd
    base_packed[:, 128:256] = np.eye(128, dtype=f)
    base_packed[:, 776:777] = b1col
    base_packed[:, 777:778] = b2col
    base_packed[:, 778:780] = b3bc
    base_packed[:, 781:785] = w3bd

    in_maps = []
    for c in range(NCORES):
        g, qh = c // 2, c % 2
        xg = np.ascontiguousarray(x[64 * g:64 * (g + 1)], dtype=f)
        pk = base_packed.copy()
        pk[0:64, 256:320] = wq[g].T
        pk[0:64, 320:384] = wq[g].T * f(DH) ** f(-0.5)
        pk[0:64, 384:448] = wk[g].T
        pk[0:64, 448:512] = wv[g].T
        pk[0:64, 512:768] = wo[:, 64 * g:64 * (g + 1)].T
        pk[0:64, 768:774] = w_off_dw
        pk[0:64, 774] = b_off_dw
        pk[0:64, 775] = 0.5 * w_off_proj
        pk[:, 780] = f(QS * qh)
        m = {
            "xg": xg,
            "xq": np.ascontiguousarray(xg[:, QS * qh:QS * (qh + 1)]),
            "mask_st": mask_st,
            "packed": pk,
        }
        in_maps.append(m)
    return in_maps


def kernel(**inputs):
    if "nc" not in _CACHED:
        _CACHED["nc"] = build_nc()
    nc = _CACHED["nc"]
    in_maps = _shard_inputs(inputs)
    res = bass_utils.run_bass_kernel_spmd(nc, in_maps, core_ids=list(range(NCORES)))
    ys = [res.results[c]["y"] for c in range(NCORES)]
    bo = inputs["bo"]
    out = np.zeros((1, DIM, N), np.float32)
    for qh in range(2):
        acc = np.zeros((DIM, QS), np.float64)
        for g in range(G):
            acc += ys[2 * g + qh]
        out[0, :, QS * qh:QS * (qh + 1)] = (acc + bo.astype(np.float64)[:, None]).astype(np.float32)
    return out



# revision 26
# speedup vs baseline: 3.3153x; 3.3153x over previous
"""DeformableAttention1D on 8 TRN2 NeuronCores via Bass/Tile.

Sharding: core c handles offset-group g=c//2 (64 of 256 channels, 2 of 8 heads)
and query-half qh=c%2 (512 of 1024 positions). Each core computes its group's
offsets/gather/CPB/attention independently; the final output projection is
computed as a partial (wo sliced by group) and summed on the host.

Key idea vs the one-hot/MLP baseline: both the grid_sample gather AND the CPB
relative-position-bias MLP are evaluated via SWDGE dma_gather from
host-precomputed DRAM tables.

  * kv gather: rows of x^T (zero-padded, pairs [x_i | x_{i+1}]) indexed by
    floor(pixel coord); bilinear lerp is 2 DVE ops with per-partition weights.
  * CPB bias: bias(q,j,o) = G_o(pos) with pos = grid_q[q] - vgs[j] and G_o a
    fixed scalar function of the CPB weights only. grid_q is a uniform grid
    with spacing delta = 2/1023, so for fixed j the 512 query positions read a
    CONTIGUOUS window of a delta-spaced table of G_o. One dma_gather of 256
    windowed rows (fp16) + a per-partition lerp replaces the whole MLP.
    (b3 is dropped: constant per (o,q) shift cancels in softmax.)

The ACT engine is restricted to ONE table set (natural_log_exp_and_others:
Exp/Ln/Relu/Copy/Identity/Square); tanh and erf(gelu) are composed from Exp.
"""
import os
import sys

sys.path.insert(0, "/opt/trn_rl_repo")

DEBUG = bool(os.environ.get("DEFORM_DEBUG"))

import numpy as np

import concourse.bacc as bacc
import concourse.bass as bass
import concourse.mybir as mybir
import concourse.tile as tile
import concourse.bass_utils as bass_utils

F32 = mybir.dt.float32
F32R = mybir.dt.float32r
F16 = mybir.dt.float16
BF16 = mybir.dt.bfloat16
I32 = mybir.dt.int32
I16 = mybir.dt.int16
U32 = mybir.dt.uint32
AF = mybir.ActivationFunctionType
ALU = mybir.AluOpType

# model dims (hardcoded per problem spec)
DIM = 256
N = 1024
G = 4
HEADS = 8
DH = 32
NDS = 256          # downsampled kv positions
QS = 512           # queries per core
DPG = 64           # channels per group
OFF_K = 6
DS = 4             # downsample stride
OFF_SCALE = 4.0
NCORES = 8

DELTA = 2.0 / 1023.0
POS0 = -2.05
K2 = 1023.0 / 255.0
WIN = 576          # CPB table window length (>= 513, mult of 64)
PMAX = 1600        # CPB windowed-table rows
TLEN = PMAX + WIN  # underlying table length
XROWS = 1059       # kv table rows (pairs), indexed by floor(ppix)+17

# A&S 7.1.26 erf coefficients (|err| <= 1.5e-7)
ERF_P = 0.3275911
ERF_A = [0.254829592, -0.284496736, 1.421413741, -1.453152027, 1.061405429]

# packed layout (f32r, [128, 900]); all matmul lhsT blocks at partitions 0:64
# (matmul requires equal base partitions for lhsT and rhs):
#   p 0:64   c 0:384    conv taps W~_k at c 64k:64k+64  (wq ∘ w_off_dw)
#   p 0:64   c 384:448  wqT (unscaled; DH^-.5 folded into wk)
#   p 0:64   c 448:512  wkT * DH^-0.5
#   p 0:64   c 512:576  wvT
#   p 0:64   c 576:832  woT (wo[:, group cols].T)
#   p 0:64   c 832:833  0.5*w_off_proj column
#   p 0:1    c 833:897  b_off_dw row (lhsT of the bias matmul)
# rowA2/rowB2 (index affine rows) ship via the separate [1,512] "rows2" input
# (DVE TSP requires equal base partitions for its two SB tensor inputs).
PK_C = 900

_CACHED = {}


def _patch_act_tables():
    """Restrict activation-table selection to the single set that covers all
    ACT functions used by this kernel, so exactly one table load is emitted."""
    import concourse.hw_specs as hw_specs

    if getattr(bacc, "_deform_act_patch", False):
        return
    orig = hw_specs.get_activation_tables

    keep = "natural_log_exp_and_others"

    def patched(module_arch):
        tabs = orig(module_arch)
        keep_funcs = tabs[keep]
        out = {}
        for name, funcs in tabs.items():
            if name == keep:
                out[name] = funcs
            else:
                out[name] = funcs - keep_funcs
        return out

    bacc.get_activation_tables = patched
    bacc._deform_act_patch = True


def build_nc():
    _patch_act_tables()
    nc = bacc.Bacc("TRN2", target_bir_lowering=False, debug=False, num_devices=NCORES)

    din = {}

    def dt_in(name, shape, dtype=F32):
        din[name] = nc.dram_tensor(name, shape, dtype, kind="ExternalInput")
        return din[name]

    dt_in("xg", [DPG, N], F32R)
    dt_in("xq", [DPG, QS], F32R)
    dt_in("packed", [128, PK_C], F32R)
    dt_in("rows2", [1, 2 * NDS], F32)
    dt_in("cpb_tab", [PMAX, 2 * WIN], F16)
    dt_in("xt2", [XROWS, 2 * DPG], F32)
    idx_scr = nc.dram_tensor("idx_scr", [1, 2 * NDS], F32, kind="Internal")
    y_out = nc.dram_tensor("y", [DIM, QS], F32, kind="ExternalOutput")
    dbg = {}
    if DEBUG:
        for nm, shp in [("dbg_conv", [DPG, NDS]), ("dbg_gl", [DPG, NDS]),
                        ("dbg_r", [1, NDS]), ("dbg_T2", [1, 2 * NDS]),
                        ("dbg_P2", [1, 2 * NDS]), ("dbg_idx", [16, 32]),
                        ("dbg_kv", [DPG, NDS]), ("dbg_k", [DPG, NDS]),
                        ("dbg_bias00", [128, QS]), ("dbg_logit00", [128, QS]),
                        ("dbg_avn", [DPG, QS])]:
            dbg[nm] = nc.dram_tensor(nm, shp, F32, kind="ExternalOutput")

    qh_off = 1  # xgp column offset of x (left zero pad)

    with tile.TileContext(nc) as tc:
        with (
            tc.tile_pool(name="const", bufs=1) as cst,
            tc.tile_pool(name="work", bufs=2) as wk,
            tc.tile_pool(name="rows", bufs=1) as rw,
            tc.tile_pool(name="pers", bufs=1) as pe_pool,
        ):
            # ---------- t=0: idle-engine prep ----------
            xgp = cst.tile([DPG, N + 4], F32R, name="xgp", tag="xgp")
            nc.gpsimd.memset(xgp[:].bitcast(F32), 0.0)
            idx16 = cst.tile([128, 32], I16, name="idx16", tag="idx16")
            # tiled identity [16, 128]: eye16[c, j] = (j % 16 == c), for
            # replicating the idx block to all 8 Q7 16-partition groups
            eyeio16 = cst.tile([16, 128], I32, name="eyeio16", tag="eyeio16")
            nc.gpsimd.iota(eyeio16[:], pattern=[[0, 8], [1, 16]], base=0,
                           channel_multiplier=-1)
            eye16 = cst.tile([16, 128], F32, name="eye16", tag="eye16")
            nc.vector.tensor_scalar(eye16[:], eyeio16[:], 0, None, ALU.is_equal)
            ones_row = cst.tile([1, NDS], F32R, name="ones_row", tag="ones_row")
            nc.gpsimd.memset(ones_row[:].bitcast(F32), 1.0)
            ones_col = cst.tile([128, 1], F32R, name="ones_col", tag="ones_col")
            nc.gpsimd.memset(ones_col[:].bitcast(F32), 1.0)
            # identity for PE transposes (f32)
            eyeio = cst.tile([128, 128], I32, name="eyeio", tag="eyeio")
            nc.gpsimd.iota(eyeio[:], pattern=[[1, 128]], base=0, channel_multiplier=-1)
            eyef = cst.tile([128, 128], F32, name="eyef", tag="eyef")
            nc.vector.tensor_scalar(eyef[:], eyeio[:], 0, None, ALU.is_equal)
            # warm the single ACT table at t=0 (overlaps input DMAs)
            wsrc = cst.tile([128, 1], F32, name="wsrc", tag="wsrc")
            nc.gpsimd.memset(wsrc[:], 0.0)
            warm = cst.tile([128, 1], F32, name="warm", tag="warm")
            nc.scalar.activation(warm[:], wsrc[:], AF.Relu)

            # ---------- input DMAs ----------
            nc.sync.dma_start(xgp[:, qh_off:qh_off + N], din["xg"].ap())
            xqt = cst.tile([DPG, QS], F32R, name="xqt", tag="xqt")
            nc.sync.dma_start(xqt[:], din["xq"].ap())
            packed = cst.tile([128, PK_C], F32R, name="packed", tag="packed")
            nc.scalar.dma_start(packed[:], din["packed"].ap())
            rows2 = cst.tile([1, 2 * NDS], F32, name="rows2", tag="rows2")
            nc.scalar.dma_start(rows2[:], din["rows2"].ap())
            wtaps = packed[0:DPG, 0:384]
            wqT = packed[0:DPG, 384:448]
            wkTs = packed[0:DPG, 448:512]
            wvT = packed[0:DPG, 512:576]
            woT = packed[0:DPG, 576:832]
            wproj = packed[0:DPG, 832:833]
            bodw_row = packed[0:1, 833:897]
            rowA2 = rows2[0:1, 0:NDS]
            rowB2 = rows2[0:1, NDS:2 * NDS]

            # persistent tiles crossing phases
            qs_sb = pe_pool.tile([DPG, QS], F32R, name="qs_sb", tag="qs_sb")
            k_sb = pe_pool.tile([DPG, NDS], F32R, name="k_sb", tag="k_sb")
            kv_sb = pe_pool.tile([DPG, NDS], F32R, name="kv_sb", tag="kv_sb")
            vT = [pe_pool.tile([128, DPG], F32R, name=f"vT{H}", tag=f"vT{H}")
                  for H in range(2)]
            fw = pe_pool.tile([128, 4], F32, name="fw", tag="fw")
            cpbg = pe_pool.tile([128, 2 * 2 * WIN], F16, name="cpbg", tag="cpbg")
            kvg = pe_pool.tile([128, 2 * 2 * DPG], F32, name="kvg", tag="kvg")
            avn = pe_pool.tile([DPG, QS], F32R, name="avn", tag="avn")

            with tc.tile_pool(name="psA", bufs=1, space="PSUM") as psA:
                # ---------- conv (strided depthwise fused with wq) ----------
                pconv = psA.tile([DPG, NDS], F32, name="pconv", tag="pconv")
                for k in range(OFF_K):
                    nc.tensor.matmul(
                        pconv[:], wtaps[:, 64 * k:64 * k + 64],
                        xgp[:, k:k + DS * (NDS - 1) + 1:DS],
                        start=(k == 0), stop=False)
                nc.tensor.matmul(pconv[:], bodw_row, ones_row[:],
                                 start=False, stop=True)
                if DEBUG:
                    dcv = wk.tile([DPG, NDS], F32, name="dcv", tag="dcv")
                    nc.vector.tensor_copy(dcv[:], pconv[:])
                    nc.sync.dma_start(dbg["dbg_conv"].ap(), dcv[:])

                # ---------- gelu: gl = 2*gelu(pconv) (0.5 folded in wproj) ----
                sq = wk.tile([DPG, NDS], F32, name="g_sq", tag="g_sq")
                nc.scalar.activation(sq[:], pconv[:], AF.Square)
                ee = wk.tile([DPG, NDS], F32, name="g_e", tag="g_e")
                nc.scalar.activation(ee[:], sq[:], AF.Exp, scale=-0.5)
                xx = wk.tile([DPG, NDS], F32, name="g_xx", tag="g_xx")
                nc.scalar.copy(xx[:], pconv[:])
                ax = wk.tile([DPG, NDS], F32, name="g_ax", tag="g_ax")
                nc.vector.scalar_tensor_tensor(ax[:], xx[:], -1.0, xx[:],
                                               ALU.mult, ALU.max)
                tt = wk.tile([DPG, NDS], F32, name="g_t", tag="g_t")
                nc.vector.tensor_scalar(tt[:], ax[:], float(ERF_P / np.sqrt(2.0)),
                                        1.0, ALU.mult, ALU.add)
                nc.vector.reciprocal(tt[:], tt[:])
                poly = wk.tile([DPG, NDS], F32, name="g_poly", tag="g_poly")
                nc.vector.tensor_scalar(poly[:], tt[:], ERF_A[4], ERF_A[3],
                                        ALU.mult, ALU.add)
                nc.vector.tensor_tensor(poly[:], poly[:], tt[:], ALU.mult)
                nc.vector.scalar_tensor_tensor(poly[:], poly[:], ERF_A[2], tt[:],
                                               ALU.add, ALU.mult)
                nc.vector.scalar_tensor_tensor(poly[:], poly[:], ERF_A[1], tt[:],
                                               ALU.add, ALU.mult)
                nc.vector.scalar_tensor_tensor(poly[:], poly[:], ERF_A[0], tt[:],
                                               ALU.add, ALU.mult)
                # A = x * P(t) * e^{-x^2/2};  2*gelu = max(2x - A, A)
                pe_ = wk.tile([DPG, NDS], F32, name="g_pe", tag="g_pe")
                nc.vector.tensor_tensor(pe_[:], poly[:], ee[:], ALU.mult)
                A_ = wk.tile([DPG, NDS], F32, name="g_A", tag="g_A")
                nc.vector.tensor_tensor(A_[:], pe_[:], pconv[:], ALU.mult)
                B_ = wk.tile([DPG, NDS], F32, name="g_B", tag="g_B")
                nc.vector.scalar_tensor_tensor(B_[:], pconv[:], 2.0, A_[:],
                                               ALU.mult, ALU.subtract)
                gl = wk.tile([DPG, NDS], F32R, name="g_gl", tag="g_gl")
                nc.vector.tensor_tensor(gl[:], B_[:], A_[:], ALU.max)
                if DEBUG:
                    nc.sync.dma_start(dbg["dbg_gl"].ap(), gl[:].bitcast(F32))

                # ---------- proj + tanh (as r = 1/(e^{2p}+1)) ----------
                pproj = psA.tile([1, NDS], F32, name="pproj", tag="pproj")
                nc.tensor.matmul(pproj[:], wproj, gl[:])
                er = rw.tile([1, NDS], F32, name="er", tag="er")
                nc.scalar.activation(er[:], pproj[:], AF.Exp, scale=2.0)
                nc.vector.tensor_scalar(er[:], er[:], 1.0, None, ALU.add)
                rr = rw.tile([1, NDS], F32, name="rr", tag="rr")
                nc.vector.reciprocal(rr[:], er[:])
                if DEBUG:
                    nc.sync.dma_start(dbg["dbg_r"].ap(), rr[:])

                # ---------- index row: u (CPB) at [0:256), ppix+17 (kv) at [256:512)
                UX = rw.tile([1, 2 * NDS], F32, name="UX", tag="UX")
                nc.vector.scalar_tensor_tensor(
                    UX[0:1, 0:NDS], rr[:], float(8.0 * K2), rowA2,
                    ALU.mult, ALU.add)
                nc.vector.scalar_tensor_tensor(
                    UX[0:1, NDS:2 * NDS], rr[:], float(-8192.0 / 255.0), rowB2,
                    ALU.mult, ALU.add)
                UXI = rw.tile([1, 2 * NDS], I32, name="UXI", tag="UXI")
                nc.vector.tensor_copy(UXI[:], UX[:])
                UXC = rw.tile([1, 2 * NDS], F32, name="UXC", tag="UXC")
                nc.vector.tensor_copy(UXC[:], UXI[:])
                GT = rw.tile([1, 2 * NDS], F32, name="GT", tag="GT")
                nc.vector.tensor_tensor(GT[:], UXC[:], UX[:], ALU.is_gt)
                P2 = rw.tile([1, 2 * NDS], F32, name="P2", tag="P2")
                nc.vector.tensor_tensor(P2[:], UXC[:], GT[:], ALU.subtract)
                F2 = rw.tile([1, 2 * NDS], F32, name="F2", tag="F2")
                nc.vector.tensor_tensor(F2[:], UX[:], P2[:], ALU.subtract)
                if DEBUG:
                    nc.sync.dma_start(dbg["dbg_T2"].ap(), UX[:])
                    nc.sync.dma_start(dbg["dbg_P2"].ap(), P2[:])

                # index row P2 (f32): [0:256)=CPB p, [256:512)=kv i0.
                # Wrap to SWDGE layout idx[p, s] = row[s*16+p] via a DRAM
                # round-trip (SBUF->SBUF partition-crossing DMA miscompiles),
                # then replicate to all 8 Q7 core groups via PE matmul.
                nc.scalar.dma_start(idx_scr.ap(), P2[:])
                idxw = rw.tile([16, 32], F32, name="idxw", tag="idxw")
                nc.scalar.dma_start(
                    idxw[:], idx_scr.ap().rearrange("a (s p) -> (a p) s", p=16))
                pidx = psA.tile([128, 32], F32, name="pidx", tag="pidx")
                nc.tensor.matmul(pidx[:], eye16[:], idxw[:])
                nc.vector.tensor_copy(idx16[:], pidx[:])
                if DEBUG:
                    didx = wk.tile([16, 32], F32, name="didx", tag="didx")
                    nc.vector.tensor_copy(didx[:], idx16[0:16, :])
                    nc.sync.dma_start(dbg["dbg_idx"].ap(), didx[:])

                # lerp weights to per-partition columns:
                # fw cols = [f_H0, w1_H0, f_H1, w1_H1]
                ptf = psA.tile([128, 4], F32, name="ptf", tag="ptf")
                for H in range(2):
                    nc.tensor.transpose(ptf[:, 2 * H:2 * H + 1],
                                        F2[0:1, 128 * H:128 * (H + 1)],
                                        eyef[0:1, 0:1])
                    nc.tensor.transpose(ptf[:, 2 * H + 1:2 * H + 2],
                                        F2[0:1, NDS + 128 * H:NDS + 128 * (H + 1)],
                                        eyef[0:1, 0:1])
                nc.scalar.copy(fw[:], ptf[:])

                # ---------- gathers (SWDGE): CPB windows first, then kv ------
                nc.gpsimd.dma_gather(
                    cpbg[:].rearrange("p (b e) -> p b e", b=2),
                    din["cpb_tab"].ap(), idx16[:, 0:16], NDS, NDS, 2 * WIN)
                nc.gpsimd.dma_gather(
                    kvg[:].rearrange("p (b e) -> p b e", b=2),
                    din["xt2"].ap(), idx16[:, 16:32], NDS, NDS, 2 * DPG)

                # ---------- qs (overlaps gathers) ----------
                pqs = psA.tile([DPG, QS], F32, name="pqs", tag="pqs")
                nc.tensor.matmul(pqs[:], wqT, xqt[:])
                nc.scalar.copy(qs_sb[:], pqs[:])

                # ---------- kv lerp + transpose + k/v ----------
                kvT = wk.tile([128, 128], F32, name="kvT", tag="kvT")
                for H in range(2):
                    b = 2 * DPG * H
                    nc.vector.tensor_tensor(
                        kvT[:, 64 * H:64 * H + 64],
                        kvg[:, b + DPG:b + 2 * DPG], kvg[:, b:b + DPG],
                        ALU.subtract)
                    nc.vector.scalar_tensor_tensor(
                        kvT[:, 64 * H:64 * H + 64],
                        kvT[:, 64 * H:64 * H + 64], fw[:, 2 * H + 1:2 * H + 2],
                        kvg[:, b:b + DPG], ALU.mult, ALU.add)
                for H in range(2):
                    pkv = psA.tile([DPG, 128], F32, name="pkv", tag="pkv")
                    nc.tensor.transpose(pkv[:], kvT[:, 64 * H:64 * H + 64],
                                        eyef[:])
                    nc.scalar.copy(kv_sb[:, 128 * H:128 * (H + 1)], pkv[:])
                if DEBUG:
                    nc.sync.dma_start(dbg["dbg_kv"].ap(), kv_sb[:].bitcast(F32))

                pk = psA.tile([DPG, NDS], F32, name="pk", tag="pk")
                nc.tensor.matmul(pk[:], wkTs, kv_sb[:])
                nc.scalar.copy(k_sb[:], pk[:])
                if DEBUG:
                    nc.sync.dma_start(dbg["dbg_k"].ap(), k_sb[:].bitcast(F32))
                for H in range(2):
                    pvT = psA.tile([128, DPG], F32, name="pvT", tag="pvT")
                    nc.tensor.matmul(pvT[:], kv_sb[:, 128 * H:128 * (H + 1)], wvT)
                    nc.scalar.copy(vT[H][:], pvT[:])

            # ---------- attention ----------
            with (
                tc.tile_pool(name="psS", bufs=2, space="PSUM") as psS,
                tc.tile_pool(name="psE", bufs=2, space="PSUM") as psE,
            ):
                psims = {}
                for h in range(2):
                    for H in range(2):
                        ps = psS.tile([128, QS], F32, name="psim", tag="psim")
                        nc.tensor.matmul(
                            ps[:], k_sb[32 * h:32 * (h + 1), 128 * H:128 * (H + 1)],
                            qs_sb[32 * h:32 * (h + 1), :])
                        psims[(h, H)] = ps

                # table holds exp(G_o - C_o); numer = exp(psim) * lerp(table)
                ets = {}
                for h in range(2):
                    o = h
                    for H in range(2):
                        base = 2 * WIN * H + WIN * o
                        R0 = cpbg[:, base:base + QS]
                        R1 = cpbg[:, base + 1:base + 1 + QS]
                        d16 = wk.tile([128, QS], F16, name="d16", tag="d16")
                        nc.vector.tensor_tensor(d16[:], R1, R0, ALU.subtract)
                        g16 = wk.tile([128, QS], F16, name="g16", tag="g16")
                        nc.vector.scalar_tensor_tensor(
                            g16[:], d16[:], fw[:, 2 * H:2 * H + 1], R0,
                            ALU.mult, ALU.add)
                        if DEBUG and h == 0 and H == 0:
                            dbb = wk.tile([128, QS], F32, name="dbb", tag="dbb")
                            nc.vector.tensor_copy(dbb[:], g16[:])
                            nc.sync.dma_start(dbg["dbg_bias00"].ap(), dbb[:])
                        ep = wk.tile([128, QS], F32, name="ep", tag="ep")
                        nc.scalar.activation(ep[:], psims[(h, H)][:], AF.Exp)
                        et = wk.tile([128, QS], F32R, name=f"et{h}{H}", tag=f"et{h}{H}")
                        nc.vector.tensor_tensor(et[:], g16[:], ep[:], ALU.mult)
                        ets[(h, H)] = et

                for h in range(2):
                    psum_s = psE.tile([1, QS], F32, name="psum_s", tag="psum_s")
                    for H in range(2):
                        nc.tensor.matmul(psum_s[:], ones_col[:], ets[(h, H)][:],
                                         start=(H == 0), stop=(H == 1))
                    rs = rw.tile([1, QS], F32, name="rs", tag=f"rs{h}")
                    nc.vector.reciprocal(rs[:], psum_s[:])
                    rsb = wk.tile([32, QS], F32, name="rsb", tag="rsb")
                    nc.gpsimd.partition_broadcast(rsb[:], rs[:])
                    pav = psE.tile([32, QS], F32, name="pav", tag="pav")
                    for H in range(2):
                        nc.tensor.matmul(pav[:], vT[H][:, 32 * h:32 * (h + 1)],
                                         ets[(h, H)][:],
                                         start=(H == 0), stop=(H == 1))
                    nc.vector.tensor_tensor(avn[32 * h:32 * (h + 1), :], pav[:],
                                            rsb[:], ALU.mult)
                if DEBUG:
                    nc.sync.dma_start(dbg["dbg_avn"].ap(), avn[:].bitcast(F32))

                # ---------- output projection ----------
                for m in range(2):
                    py = psS.tile([128, QS], F32, name="py", tag="py")
                    nc.tensor.matmul(
                        py[:], woT[:, 128 * m:128 * (m + 1)], avn[:])
                    y_sb = wk.tile([128, QS], F32, name="y_sb", tag="y_sb")
                    nc.scalar.copy(y_sb[:], py[:])
                    nc.sync.dma_start(y_out.ap()[128 * m:128 * (m + 1), :], y_sb[:])

    nc.compile()
    return nc


def _build_cpb_table(w1, b1, w2, b2, w3):
    """Windowed fp16 table of exp(G_o(pos) - C_o) on the delta grid (the
    per-o shift C_o cancels in softmax; exp-space lets the bias apply as a
    multiply after exp(sim)). Returns [PMAX, 2*WIN] fp16."""
    m = np.arange(TLEN, dtype=np.float64)
    pos = POS0 + m * (2.0 / 1023.0)
    t = np.sign(pos) * np.log1p(np.abs(pos))
    H1 = np.maximum(t[:, None] * w1[None, :] + b1[None, :], 0.0)
    H2 = np.maximum(H1 @ w2.T + b2[None, :], 0.0)
    B = H2 @ w3.T                                        # [TLEN, 2] (b3 dropped)
    E = np.exp(B - B.max(axis=0, keepdims=True))
    E = np.maximum(E, 6.2e-5)   # keep fp16 normal; only where attn weight ~0
    sw = np.lib.stride_tricks.sliding_window_view(E, WIN, axis=0)  # [TLEN-WIN+1, 2, WIN]
    return np.ascontiguousarray(
        sw[:PMAX].reshape(PMAX, 2 * WIN)).astype(np.float16)


def _shard_inputs(inputs):
    x = np.ascontiguousarray(inputs["x"][0]).astype(np.float32)   # [256, 1024]
    wq, wk, wv = inputs["wq"], inputs["wk"], inputs["wv"]
    wo = inputs["wo"]
    w_off_dw = inputs["w_off_dw"][:, 0, :]                 # [64, 6]
    b_off_dw = inputs["b_off_dw"]
    w_off_proj = inputs["w_off_proj"]
    w1 = np.asarray(inputs["cpb_w1"][:, 0], np.float64)
    b1 = np.asarray(inputs["cpb_b1"], np.float64)
    w2 = np.asarray(inputs["cpb_w2"], np.float64)
    b2 = np.asarray(inputs["cpb_b2"], np.float64)
    w3 = np.asarray(inputs["cpb_w3"], np.float64)

    f = np.float32
    cpb_tab = _build_cpb_table(w1, b1, w2, b2, w3)

    j = np.arange(NDS, dtype=np.float64)
    rowB2 = (1024.0 / 255.0) * j + 16.5 + 4096.0 / 255.0

    in_maps = []
    for c in range(NCORES):
        g, qh = c // 2, c % 2
        xg = np.ascontiguousarray(x[64 * g:64 * (g + 1)], dtype=f)
        qbase = float(QS * qh)
        rowA2 = qbase - POS0 * 511.5 - K2 * j - 4.0 * K2

        pk = np.zeros((128, PK_C), f)
        for k in range(OFF_K):
            pk[0:64, 64 * k:64 * k + 64] = wq[g].T * w_off_dw[None, :, k]
        pk[0:64, 384:448] = wq[g].T
        pk[0:64, 448:512] = wk[g].T * f(DH) ** f(-0.5)
        pk[0:64, 512:576] = wv[g].T
        pk[0:64, 576:832] = wo[:, 64 * g:64 * (g + 1)].T
        pk[0:64, 832] = 0.5 * w_off_proj
        pk[0, 833:897] = b_off_dw
        pk[64, 0:256] = rowA2
        pk[96, 256:512] = rowB2

        xpad = np.zeros((XROWS + 1, DPG), f)
        xpad[17:17 + N] = xg.T
        xt2 = np.concatenate([xpad[:-1], xpad[1:]], axis=1)  # [1059, 128]

        rows2 = np.concatenate([rowA2, rowB2]).astype(f)[None, :]
        in_maps.append({
            "xg": xg,
            "rows2": rows2,
            "xq": np.ascontiguousarray(xg[:, QS * qh:QS * (qh + 1)]),
            "packed": pk,
            "cpb_tab": cpb_tab,
            "xt2": np.ascontiguousarray(xt2),
        })
    return in_maps


def kernel(**inputs):
    if "nc" not in _CACHED:
        _CACHED["nc"] = build_nc()
    nc = _CACHED["nc"]
    in_maps = _shard_inputs(inputs)
    res = bass_utils.run_bass_kernel_spmd(nc, in_maps, core_ids=list(range(NCORES)))
    ys = [res.results[c]["y"] for c in range(NCORES)]
    bo = inputs["bo"]
    out = np.zeros((1, DIM, N), np.float32)
    for qh in range(2):
        acc = np.zeros((DIM, QS), np.float64)
        for g in range(G):
            acc += ys[2 * g + qh]
        out[0, :, QS * qh:QS * (qh + 1)] = (
            acc + bo.astype(np.float64)[:, None]).astype(np.float32)
    return out


# revision 31
# speedup vs baseline: 3.4793x; 1.0495x over previous
"""DeformableAttention1D on 8 TRN2 NeuronCores via Bass/Tile.

Sharding: core c handles offset-group g=c//2 (64 of 256 channels, 2 of 8 heads)
and query-half qh=c%2 (512 of 1024 positions). Each core computes its group's
offsets/gather/CPB/attention independently; the final output projection is
computed as a partial (wo sliced by group) and summed on the host.

Key idea vs the one-hot/MLP baseline: both the grid_sample gather AND the CPB
relative-position-bias MLP are evaluated via SWDGE dma_gather from
host-precomputed DRAM tables.

  * kv gather: rows of x^T (zero-padded, pairs [x_i | x_{i+1}]) indexed by
    floor(pixel coord); bilinear lerp is 2 DVE ops with per-partition weights.
  * CPB bias: bias(q,j,o) = G_o(pos) with pos = grid_q[q] - vgs[j] and G_o a
    fixed scalar function of the CPB weights only. grid_q is a uniform grid
    with spacing delta = 2/1023, so for fixed j the 512 query positions read a
    CONTIGUOUS window of a delta-spaced table of G_o. One dma_gather of 256
    windowed rows (fp16) + a per-partition lerp replaces the whole MLP.
    (b3 is dropped: constant per (o,q) shift cancels in softmax.)

The ACT engine is restricted to ONE table set (natural_log_exp_and_others:
Exp/Ln/Relu/Copy/Identity/Square); tanh and erf(gelu) are composed from Exp.
"""
import os
import sys

sys.path.insert(0, "/opt/trn_rl_repo")

DEBUG = bool(os.environ.get("DEFORM_DEBUG"))

import numpy as np

import concourse.bacc as bacc
import concourse.bass as bass
import concourse.mybir as mybir
import concourse.tile as tile
import concourse.bass_utils as bass_utils

F32 = mybir.dt.float32
F32R = mybir.dt.float32r
F16 = mybir.dt.float16
BF16 = mybir.dt.bfloat16
I32 = mybir.dt.int32
I16 = mybir.dt.int16
U32 = mybir.dt.uint32
AF = mybir.ActivationFunctionType
ALU = mybir.AluOpType

# model dims (hardcoded per problem spec)
DIM = 256
N = 1024
G = 4
HEADS = 8
DH = 32
NDS = 256          # downsampled kv positions
QS = 512           # queries per core
DPG = 64           # channels per group
OFF_K = 6
DS = 4             # downsample stride
OFF_SCALE = 4.0
NCORES = 8

DELTA = 2.0 / 1023.0
POS0 = -2.05
K2 = 1023.0 / 255.0
WIN = 576          # CPB table window length (>= 513, mult of 64)
PMAX = 1600        # CPB windowed-table rows
TLEN = PMAX + WIN  # underlying table length
XROWS = 1059       # kv table rows (pairs), indexed by floor(ppix)+17

# A&S 7.1.26 erf coefficients (|err| <= 1.5e-7)
ERF_P = 0.3275911
ERF_A = [0.254829592, -0.284496736, 1.421413741, -1.453152027, 1.061405429]

# packed layout (f32r, [128, 900]); all matmul lhsT blocks at partitions 0:64
# (matmul requires equal base partitions for lhsT and rhs):
#   p 0:64   c 0:384    conv taps W~_k at c 64k:64k+64  (wq ∘ w_off_dw)
#   p 0:64   c 384:448  wqT (unscaled; DH^-.5 folded into wk)
#   p 0:64   c 448:512  wkT * DH^-0.5
#   p 0:64   c 512:576  wvT
#   p 0:64   c 576:832  woT (wo[:, group cols].T)
#   p 0:64   c 832:833  0.5*w_off_proj column
#   p 0:1    c 833:897  b_off_dw row (lhsT of the bias matmul)
# rowA2/rowB2 (index affine rows) ship via the separate [1,512] "rows2" input
# (DVE TSP requires equal base partitions for its two SB tensor inputs).
PK_C = 900

_CACHED = {}


def _patch_act_tables():
    """Restrict activation-table selection to the single set that covers all
    ACT functions used by this kernel, so exactly one table load is emitted."""
    import concourse.hw_specs as hw_specs

    if getattr(bacc, "_deform_act_patch", False):
        return
    orig = hw_specs.get_activation_tables

    keep = "natural_log_exp_and_others"

    def patched(module_arch):
        tabs = orig(module_arch)
        keep_funcs = tabs[keep]
        out = {}
        for name, funcs in tabs.items():
            if name == keep:
                out[name] = funcs
            else:
                out[name] = funcs - keep_funcs
        return out

    bacc.get_activation_tables = patched
    bacc._deform_act_patch = True


def build_nc():
    _patch_act_tables()
    nc = bacc.Bacc("TRN2", target_bir_lowering=False, debug=False, num_devices=NCORES)

    din = {}

    def dt_in(name, shape, dtype=F32):
        din[name] = nc.dram_tensor(name, shape, dtype, kind="ExternalInput")
        return din[name]

    dt_in("xg", [DPG, N], F32R)
    dt_in("xq", [DPG, QS], F32R)
    dt_in("packed", [128, PK_C], F32R)
    dt_in("rows2", [1, 2 * NDS], F32)
    dt_in("cpb_tab", [PMAX, 2 * WIN], F16)
    dt_in("xt2", [XROWS, 2 * DPG], F32)
    idx_scr = nc.dram_tensor("idx_scr", [1, 2 * NDS], F32, kind="Internal")
    y_out = nc.dram_tensor("y", [DIM, QS], F16, kind="ExternalOutput")
    dbg = {}
    if DEBUG:
        for nm, shp in [("dbg_conv", [DPG, NDS]), ("dbg_gl", [DPG, NDS]),
                        ("dbg_r", [1, NDS]), ("dbg_T2", [1, 2 * NDS]),
                        ("dbg_P2", [1, 2 * NDS]), ("dbg_idx", [16, 32]),
                        ("dbg_kv", [DPG, NDS]), ("dbg_k", [DPG, NDS]),
                        ("dbg_bias00", [128, QS]), ("dbg_logit00", [128, QS]),
                        ("dbg_avn", [DPG, QS])]:
            dbg[nm] = nc.dram_tensor(nm, shp, F32, kind="ExternalOutput")

    qh_off = 1  # xgp column offset of x (left zero pad)

    with tile.TileContext(nc) as tc:
        with (
            tc.tile_pool(name="const", bufs=1) as cst,
            tc.tile_pool(name="work", bufs=2) as wk,
            tc.tile_pool(name="rows", bufs=1) as rw,
            tc.tile_pool(name="pers", bufs=1) as pe_pool,
        ):
            # ---------- t=0: idle-engine prep ----------
            xgp = cst.tile([DPG, N + 4], F32R, name="xgp", tag="xgp")
            nc.gpsimd.memset(xgp[:].bitcast(F32), 0.0)
            idx16 = cst.tile([128, 32], I16, name="idx16", tag="idx16")
            # tiled identity [16, 128]: eye16[c, j] = (j % 16 == c), for
            # replicating the idx block to all 8 Q7 16-partition groups
            eyeio16 = cst.tile([16, 128], I32, name="eyeio16", tag="eyeio16")
            nc.gpsimd.iota(eyeio16[:], pattern=[[0, 8], [1, 16]], base=0,
                           channel_multiplier=-1)
            eye16 = cst.tile([16, 128], F32, name="eye16", tag="eye16")
            nc.vector.tensor_scalar(eye16[:], eyeio16[:], 0, None, ALU.is_equal)
            ones_row = cst.tile([1, NDS], F32R, name="ones_row", tag="ones_row")
            nc.gpsimd.memset(ones_row[:].bitcast(F32), 1.0)
            ones_col = cst.tile([128, 1], F32R, name="ones_col", tag="ones_col")
            nc.gpsimd.memset(ones_col[:].bitcast(F32), 1.0)
            # identity for PE transposes (f32)
            eyeio = cst.tile([128, 128], I32, name="eyeio", tag="eyeio")
            nc.gpsimd.iota(eyeio[:], pattern=[[1, 128]], base=0, channel_multiplier=-1)
            eyef = cst.tile([128, 128], F32, name="eyef", tag="eyef")
            nc.vector.tensor_scalar(eyef[:], eyeio[:], 0, None, ALU.is_equal)
            # warm the single ACT table at t=0 (overlaps input DMAs)
            wsrc = cst.tile([128, 1], F32, name="wsrc", tag="wsrc")
            nc.gpsimd.memset(wsrc[:], 0.0)
            warm = cst.tile([128, 1], F32, name="warm", tag="warm")
            nc.scalar.activation(warm[:], wsrc[:], AF.Relu)

            # ---------- input DMAs (packed on ACT queue; rest on SP) ----
            packed = cst.tile([128, PK_C], F32R, name="packed", tag="packed")
            nc.scalar.dma_start(packed[:], din["packed"].ap())
            nc.sync.dma_start(xgp[:, qh_off:qh_off + N], din["xg"].ap())
            xqt = cst.tile([DPG, QS], F32R, name="xqt", tag="xqt")
            nc.sync.dma_start(xqt[:], din["xq"].ap())
            rows2 = cst.tile([1, 2 * NDS], F32, name="rows2", tag="rows2")
            nc.sync.dma_start(rows2[:], din["rows2"].ap())
            wtaps = packed[0:DPG, 0:384]
            wqT = packed[0:DPG, 384:448]
            wkTs = packed[0:DPG, 448:512]
            wvT = packed[0:DPG, 512:576]
            woT = packed[0:DPG, 576:832]
            wproj = packed[0:DPG, 832:833]
            bodw_row = packed[0:1, 833:897]
            rowA2 = rows2[0:1, 0:NDS]
            rowB2 = rows2[0:1, NDS:2 * NDS]

            # persistent tiles crossing phases
            qs_sb = pe_pool.tile([DPG, QS], F32R, name="qs_sb", tag="qs_sb")
            k_sb = pe_pool.tile([DPG, NDS], F32R, name="k_sb", tag="k_sb")
            kv_sb = pe_pool.tile([DPG, NDS], F32R, name="kv_sb", tag="kv_sb")
            vT = [pe_pool.tile([128, DPG], F32R, name=f"vT{H}", tag=f"vT{H}")
                  for H in range(2)]
            fw = pe_pool.tile([128, 4], F32, name="fw", tag="fw")
            cpbg = pe_pool.tile([128, 2 * 2 * WIN], F16, name="cpbg", tag="cpbg")
            kvg = pe_pool.tile([128, 2 * 2 * DPG], F32, name="kvg", tag="kvg")
            avn = pe_pool.tile([DPG, QS], F32R, name="avn", tag="avn")

            with (
                tc.tile_pool(name="psA", bufs=1, space="PSUM") as psA,
                tc.tile_pool(name="psB", bufs=1, space="PSUM") as psB,
            ):
                # ---------- conv (strided depthwise fused with wq) ----------
                pconv = psA.tile([DPG, NDS], F32, name="pconv", tag="pconv")
                for k in range(OFF_K):
                    nc.tensor.matmul(
                        pconv[:], wtaps[:, 64 * k:64 * k + 64],
                        xgp[:, k:k + DS * (NDS - 1) + 1:DS],
                        start=(k == 0), stop=False)
                nc.tensor.matmul(pconv[:], bodw_row, ones_row[:],
                                 start=False, stop=True)
                if DEBUG:
                    dcv = wk.tile([DPG, NDS], F32, name="dcv", tag="dcv")
                    nc.vector.tensor_copy(dcv[:], pconv[:])
                    nc.sync.dma_start(dbg["dbg_conv"].ap(), dcv[:])

                # ---------- gelu: gl = 2*gelu(pconv), A&S 7.1.25 3-term ------
                sq = wk.tile([DPG, NDS], F32, name="g_sq", tag="g_sq")
                nc.scalar.activation(sq[:], pconv[:], AF.Square)
                ee = wk.tile([DPG, NDS], F32, name="g_e", tag="g_e")
                nc.scalar.activation(ee[:], sq[:], AF.Exp, scale=-0.5)
                xx = wk.tile([DPG, NDS], F32, name="g_xx", tag="g_xx")
                nc.scalar.copy(xx[:], pconv[:])
                ax = wk.tile([DPG, NDS], F32, name="g_ax", tag="g_ax")
                nc.vector.scalar_tensor_tensor(ax[:], xx[:], -1.0, xx[:],
                                               ALU.mult, ALU.max)
                # t = 1/(1 + p*|x|/sqrt2) -- the affine part on ACT
                tp = wk.tile([DPG, NDS], F32, name="g_tp", tag="g_tp")
                nc.scalar.activation(tp[:], ax[:], AF.Copy,
                                     scale=float(0.47047 / np.sqrt(2.0)), bias=1.0)
                tt = wk.tile([DPG, NDS], F32, name="g_t", tag="g_t")
                nc.vector.reciprocal(tt[:], tp[:])
                # P(t)*t = ((a3 t + a2) t + a1) t
                poly = wk.tile([DPG, NDS], F32, name="g_poly", tag="g_poly")
                nc.vector.tensor_scalar(poly[:], tt[:], 0.7478556, -0.0958798,
                                        ALU.mult, ALU.add)
                nc.vector.tensor_tensor(poly[:], poly[:], tt[:], ALU.mult)
                nc.vector.scalar_tensor_tensor(poly[:], poly[:], 0.3480242, tt[:],
                                               ALU.add, ALU.mult)
                # A = x * P(t)t * e^{-x^2/2};  2*gelu = max(2x - A, A)
                pe_ = wk.tile([DPG, NDS], F32, name="g_pe", tag="g_pe")
                nc.vector.tensor_tensor(pe_[:], poly[:], ee[:], ALU.mult)
                A_ = wk.tile([DPG, NDS], F32, name="g_A", tag="g_A")
                nc.vector.tensor_tensor(A_[:], pe_[:], pconv[:], ALU.mult)
                B_ = wk.tile([DPG, NDS], F32, name="g_B", tag="g_B")
                nc.vector.scalar_tensor_tensor(B_[:], pconv[:], 2.0, A_[:],
                                               ALU.mult, ALU.subtract)
                gl = wk.tile([DPG, NDS], F32R, name="g_gl", tag="g_gl")
                nc.vector.tensor_tensor(gl[:], B_[:], A_[:], ALU.max)
                if DEBUG:
                    nc.sync.dma_start(dbg["dbg_gl"].ap(), gl[:].bitcast(F32))

                # ---------- proj + tanh (as r = 1/(e^{2p}+1)) ----------
                pproj = psA.tile([1, NDS], F32, name="pproj", tag="pproj")
                nc.tensor.matmul(pproj[:], wproj, gl[:])
                # qs early on PE (data ready; overlaps the row chain)
                pqs = psA.tile([DPG, QS], F32, name="pqs", tag="pqs")
                nc.tensor.matmul(pqs[:], wqT, xqt[:])
                nc.scalar.copy(qs_sb[:], pqs[:])

                er = rw.tile([1, NDS], F32, name="er", tag="er")
                nc.scalar.activation(er[:], pproj[:], AF.Exp, scale=2.0)
                er1 = rw.tile([1, NDS], F32, name="er1", tag="er1")
                nc.scalar.activation(er1[:], er[:], AF.Copy, bias=1.0)
                rr = rw.tile([1, NDS], F32, name="rr", tag="rr")
                nc.vector.reciprocal(rr[:], er1[:])
                if DEBUG:
                    nc.sync.dma_start(dbg["dbg_r"].ap(), rr[:])

                # ---------- index row: u (CPB) at [0:256), ppix+17 at [256:512)
                UX = rw.tile([1, 2 * NDS], F32, name="UX", tag="UX")
                nc.vector.scalar_tensor_tensor(
                    UX[0:1, 0:NDS], rr[:], float(8.0 * K2), rowA2,
                    ALU.mult, ALU.add)
                nc.vector.scalar_tensor_tensor(
                    UX[0:1, NDS:2 * NDS], rr[:], float(-8192.0 / 255.0), rowB2,
                    ALU.mult, ALU.add)
                UXI = rw.tile([1, 2 * NDS], I32, name="UXI", tag="UXI")
                nc.vector.tensor_copy(UXI[:], UX[:])
                UXC = rw.tile([1, 2 * NDS], F32, name="UXC", tag="UXC")
                nc.vector.tensor_copy(UXC[:], UXI[:])
                GT = rw.tile([1, 2 * NDS], F32, name="GT", tag="GT")
                nc.vector.tensor_tensor(GT[:], UXC[:], UX[:], ALU.is_gt)
                P2 = rw.tile([1, 2 * NDS], F32, name="P2", tag="P2")
                nc.vector.tensor_tensor(P2[:], UXC[:], GT[:], ALU.subtract)
                # fire the wrap round-trip ASAP (SP queue), THEN compute fracs
                nc.sync.dma_start(idx_scr.ap(), P2[:])
                idxw = rw.tile([16, 32], F32, name="idxw", tag="idxw")
                nc.sync.dma_start(
                    idxw[:], idx_scr.ap().rearrange("a (s p) -> (a p) s", p=16))
                F2 = rw.tile([1, 2 * NDS], F32, name="F2", tag="F2")
                nc.vector.tensor_tensor(F2[:], UX[:], P2[:], ALU.subtract)
                if DEBUG:
                    nc.sync.dma_start(dbg["dbg_T2"].ap(), UX[:])
                    nc.sync.dma_start(dbg["dbg_P2"].ap(), P2[:])

                # lerp weights to per-partition columns:
                # fw cols = [f_H0, w1_H0, f_H1, w1_H1]
                ptf = psA.tile([128, 4], F32, name="ptf", tag="ptf")
                for H in range(2):
                    nc.tensor.transpose(ptf[:, 2 * H:2 * H + 1],
                                        F2[0:1, 128 * H:128 * (H + 1)],
                                        eyef[0:1, 0:1])
                    nc.tensor.transpose(ptf[:, 2 * H + 1:2 * H + 2],
                                        F2[0:1, NDS + 128 * H:NDS + 128 * (H + 1)],
                                        eyef[0:1, 0:1])
                nc.scalar.copy(fw[:], ptf[:])

                # replicate idx block to all 8 Q7 core groups via PE matmul
                pidx = psA.tile([128, 32], F32, name="pidx", tag="pidx")
                nc.tensor.matmul(pidx[:], eye16[:], idxw[:])
                nc.vector.tensor_copy(idx16[:], pidx[:])
                if DEBUG:
                    didx = wk.tile([16, 32], F32, name="didx", tag="didx")
                    nc.vector.tensor_copy(didx[:], idx16[0:16, :])
                    nc.sync.dma_start(dbg["dbg_idx"].ap(), didx[:])

                # ---------- gathers (SWDGE): kv first (unblocks k/v/psim) ----
                nc.gpsimd.dma_gather(
                    kvg[:].rearrange("p (b e) -> p b e", b=2),
                    din["xt2"].ap(), idx16[:, 16:32], NDS, NDS, 2 * DPG)
                nc.gpsimd.dma_gather(
                    cpbg[:].rearrange("p (b e) -> p b e", b=2),
                    din["cpb_tab"].ap(), idx16[:, 0:16], NDS, NDS, 2 * WIN)

                # ---------- kv lerp + transpose + k/v ----------
                kvT = wk.tile([128, 128], F32, name="kvT", tag="kvT")
                for H in range(2):
                    b = 2 * DPG * H
                    nc.vector.tensor_tensor(
                        kvT[:, 64 * H:64 * H + 64],
                        kvg[:, b + DPG:b + 2 * DPG], kvg[:, b:b + DPG],
                        ALU.subtract)
                    nc.vector.scalar_tensor_tensor(
                        kvT[:, 64 * H:64 * H + 64],
                        kvT[:, 64 * H:64 * H + 64], fw[:, 2 * H + 1:2 * H + 2],
                        kvg[:, b:b + DPG], ALU.mult, ALU.add)
                for H in range(2):
                    pkv = psB.tile([DPG, 128], F32, name="pkv", tag="pkv")
                    nc.tensor.transpose(pkv[:], kvT[:, 64 * H:64 * H + 64],
                                        eyef[:])
                    nc.scalar.copy(kv_sb[:, 128 * H:128 * (H + 1)], pkv[:])
                if DEBUG:
                    nc.sync.dma_start(dbg["dbg_kv"].ap(), kv_sb[:].bitcast(F32))

                pk = psA.tile([DPG, NDS], F32, name="pk", tag="pk")
                nc.tensor.matmul(pk[:], wkTs, kv_sb[:])
                nc.scalar.copy(k_sb[:], pk[:])
                if DEBUG:
                    nc.sync.dma_start(dbg["dbg_k"].ap(), k_sb[:].bitcast(F32))
                for H in range(2):
                    pvT = psB.tile([128, DPG], F32, name="pvT", tag="pvT")
                    nc.tensor.matmul(pvT[:], kv_sb[:, 128 * H:128 * (H + 1)], wvT)
                    nc.scalar.copy(vT[H][:], pvT[:])

            # ---------- attention ----------
            with (
                tc.tile_pool(name="psS", bufs=2, space="PSUM") as psS,
                tc.tile_pool(name="psY", bufs=1, space="PSUM") as psY,
                tc.tile_pool(name="psE", bufs=2, space="PSUM") as psE,
            ):
                psims = {}
                eps = {}
                for h in range(2):
                    for H in range(2):
                        ps = psS.tile([128, QS], F32, name="psim", tag="psim")
                        nc.tensor.matmul(
                            ps[:], k_sb[32 * h:32 * (h + 1), 128 * H:128 * (H + 1)],
                            qs_sb[32 * h:32 * (h + 1), :])
                        psims[(h, H)] = ps
                        # exp(sim) on ACT right away (PSUM -> SBUF)
                        ep = wk.tile([128, QS], F32, name="ep", tag="ep")
                        nc.scalar.activation(ep[:], ps[:], AF.Exp)
                        eps[(h, H)] = ep

                # table holds exp(G_o - C_o); numer = exp(psim) * lerp(table)
                ets = {}
                for h in range(2):
                    o = h
                    for H in range(2):
                        base = 2 * WIN * H + WIN * o
                        R0 = cpbg[:, base:base + QS]
                        R1 = cpbg[:, base + 1:base + 1 + QS]
                        d16 = wk.tile([128, QS], F16, name="d16", tag="d16")
                        nc.vector.tensor_tensor(d16[:], R1, R0, ALU.subtract)
                        g16 = wk.tile([128, QS], F16, name="g16", tag="g16")
                        nc.vector.scalar_tensor_tensor(
                            g16[:], d16[:], fw[:, 2 * H:2 * H + 1], R0,
                            ALU.mult, ALU.add)
                        if DEBUG and h == 0 and H == 0:
                            dbb = wk.tile([128, QS], F32, name="dbb", tag="dbb")
                            nc.vector.tensor_copy(dbb[:], g16[:])
                            nc.sync.dma_start(dbg["dbg_bias00"].ap(), dbb[:])
                        et = wk.tile([128, QS], F32R, name=f"et{h}{H}",
                                     tag=f"et{h}{H}")
                        nc.vector.tensor_tensor(et[:], g16[:], eps[(h, H)][:],
                                                ALU.mult)
                        ets[(h, H)] = et

                # softmax denominators + weighted values, interleaved across h
                psums, pavs = {}, {}
                for h in range(2):
                    psum_s = psE.tile([1, QS], F32, name="psum_s", tag="psum_s")
                    for H in range(2):
                        nc.tensor.matmul(psum_s[:], ones_col[:], ets[(h, H)][:],
                                         start=(H == 0), stop=(H == 1))
                    psums[h] = psum_s
                    pav = psE.tile([32, QS], F32, name="pav", tag="pav")
                    for H in range(2):
                        nc.tensor.matmul(pav[:], vT[H][:, 32 * h:32 * (h + 1)],
                                         ets[(h, H)][:],
                                         start=(H == 0), stop=(H == 1))
                    pavs[h] = pav
                for h in range(2):
                    rs = rw.tile([1, QS], F32, name="rs", tag=f"rs{h}")
                    nc.vector.reciprocal(rs[:], psums[h][:])
                    rsb = wk.tile([32, QS], F32, name="rsb", tag="rsb")
                    nc.gpsimd.partition_broadcast(rsb[:], rs[:])
                    nc.vector.tensor_tensor(avn[32 * h:32 * (h + 1), :],
                                            pavs[h][:], rsb[:], ALU.mult)
                if DEBUG:
                    nc.sync.dma_start(dbg["dbg_avn"].ap(), avn[:].bitcast(F32))

                # ---------- output projection (h-split accumulation so py
                # starts right after head 0's avn; fp16 output halves the DMA)
                pys = [psY.tile([128, QS], F32, name=f"py{m}", tag=f"py{m}")
                       for m in range(2)]
                for h in range(2):
                    for m in range(2):
                        nc.tensor.matmul(
                            pys[m][:],
                            woT[32 * h:32 * (h + 1), 128 * m:128 * (m + 1)],
                            avn[32 * h:32 * (h + 1), :],
                            start=(h == 0), stop=(h == 1))
                for m in range(2):
                    y_sb = wk.tile([128, QS], F16, name="y_sb", tag="y_sb")
                    nc.scalar.copy(y_sb[:], pys[m][:])
                    nc.sync.dma_start(y_out.ap()[128 * m:128 * (m + 1), :], y_sb[:])

    nc.compile()
    return nc


def _build_cpb_table(w1, b1, w2, b2, w3):
    """Windowed fp16 table of exp(G_o(pos) - C_o) on the delta grid (the
    per-o shift C_o cancels in softmax; exp-space lets the bias apply as a
    multiply after exp(sim)). Returns [PMAX, 2*WIN] fp16."""
    m = np.arange(TLEN, dtype=np.float64)
    pos = POS0 + m * (2.0 / 1023.0)
    t = np.sign(pos) * np.log1p(np.abs(pos))
    H1 = np.maximum(t[:, None] * w1[None, :] + b1[None, :], 0.0)
    H2 = np.maximum(H1 @ w2.T + b2[None, :], 0.0)
    B = H2 @ w3.T                                        # [TLEN, 2] (b3 dropped)
    E = np.exp(B - B.max(axis=0, keepdims=True))
    E = np.maximum(E, 6.2e-5)   # keep fp16 normal; only where attn weight ~0
    sw = np.lib.stride_tricks.sliding_window_view(E, WIN, axis=0)  # [TLEN-WIN+1, 2, WIN]
    return np.ascontiguousarray(
        sw[:PMAX].reshape(PMAX, 2 * WIN)).astype(np.float16)


def _shard_inputs(inputs):
    x = np.ascontiguousarray(inputs["x"][0]).astype(np.float32)   # [256, 1024]
    wq, wk, wv = inputs["wq"], inputs["wk"], inputs["wv"]
    wo = inputs["wo"]
    w_off_dw = inputs["w_off_dw"][:, 0, :]                 # [64, 6]
    b_off_dw = inputs["b_off_dw"]
    w_off_proj = inputs["w_off_proj"]
    w1 = np.asarray(inputs["cpb_w1"][:, 0], np.float64)
    b1 = np.asarray(inputs["cpb_b1"], np.float64)
    w2 = np.asarray(inputs["cpb_w2"], np.float64)
    b2 = np.asarray(inputs["cpb_b2"], np.float64)
    w3 = np.asarray(inputs["cpb_w3"], np.float64)

    f = np.float32
    cpb_tab = _build_cpb_table(w1, b1, w2, b2, w3)

    j = np.arange(NDS, dtype=np.float64)
    rowB2 = (1024.0 / 255.0) * j + 16.5 + 4096.0 / 255.0

    in_maps = []
    for c in range(NCORES):
        g, qh = c // 2, c % 2
        xg = np.ascontiguousarray(x[64 * g:64 * (g + 1)], dtype=f)
        qbase = float(QS * qh)
        rowA2 = qbase - POS0 * 511.5 - K2 * j - 4.0 * K2

        pk = np.zeros((128, PK_C), f)
        for k in range(OFF_K):
            pk[0:64, 64 * k:64 * k + 64] = wq[g].T * w_off_dw[None, :, k]
        pk[0:64, 384:448] = wq[g].T
        pk[0:64, 448:512] = wk[g].T * f(DH) ** f(-0.5)
        pk[0:64, 512:576] = wv[g].T
        pk[0:64, 576:832] = wo[:, 64 * g:64 * (g + 1)].T
        pk[0:64, 832] = 0.5 * w_off_proj
        pk[0, 833:897] = b_off_dw
        pk[64, 0:256] = rowA2
        pk[96, 256:512] = rowB2

        xpad = np.zeros((XROWS + 1, DPG), f)
        xpad[17:17 + N] = xg.T
        xt2 = np.concatenate([xpad[:-1], xpad[1:]], axis=1)  # [1059, 128]

        rows2 = np.concatenate([rowA2, rowB2]).astype(f)[None, :]
        in_maps.append({
            "xg": xg,
            "rows2": rows2,
            "xq": np.ascontiguousarray(xg[:, QS * qh:QS * (qh + 1)]),
            "packed": pk,
            "cpb_tab": cpb_tab,
            "xt2": np.ascontiguousarray(xt2),
        })
    return in_maps


def kernel(**inputs):
    if "nc" not in _CACHED:
        _CACHED["nc"] = build_nc()
    nc = _CACHED["nc"]
    in_maps = _shard_inputs(inputs)
    res = bass_utils.run_bass_kernel_spmd(nc, in_maps, core_ids=list(range(NCORES)))
    ys = [res.results[c]["y"] for c in range(NCORES)]
    bo = inputs["bo"]
    out = np.zeros((1, DIM, N), np.float32)
    for qh in range(2):
        acc = np.zeros((DIM, QS), np.float64)
        for g in range(G):
            acc += ys[2 * g + qh]
        out[0, :, QS * qh:QS * (qh + 1)] = (
            acc + bo.astype(np.float64)[:, None]).astype(np.float32)
    return out


# revision 35
# speedup vs baseline: 3.8487x; 1.1062x over previous
"""DeformableAttention1D on 8 TRN2 NeuronCores via Bass/Tile.

Sharding: core c handles offset-group g=c//2 (64 of 256 channels, 2 of 8 heads)
and query-half qh=c%2 (512 of 1024 positions). Each core computes its group's
offsets/gather/CPB/attention independently; the final output projection is
computed as a partial (wo sliced by group) and summed on the host.

Key idea vs the one-hot/MLP baseline: both the grid_sample gather AND the CPB
relative-position-bias MLP are evaluated via SWDGE dma_gather from
host-precomputed DRAM tables.

  * kv gather: rows of x^T (zero-padded, pairs [x_i | x_{i+1}]) indexed by
    floor(pixel coord); bilinear lerp is 2 DVE ops with per-partition weights.
  * CPB bias: bias(q,j,o) = G_o(pos) with pos = grid_q[q] - vgs[j] and G_o a
    fixed scalar function of the CPB weights only. grid_q is a uniform grid
    with spacing delta = 2/1023, so for fixed j the 512 query positions read a
    CONTIGUOUS window of a delta-spaced table of G_o. One dma_gather of 256
    windowed rows (fp16) + a per-partition lerp replaces the whole MLP.
    (b3 is dropped: constant per (o,q) shift cancels in softmax.)

The ACT engine is restricted to ONE table set (exp_and_others: Exp, Tanh,
Square, Copy, Relu, ...); gelu uses the tanh approximation natively.
"""
import os
import sys

sys.path.insert(0, "/opt/trn_rl_repo")

DEBUG = bool(os.environ.get("DEFORM_DEBUG"))

import numpy as np

import concourse.bacc as bacc
import concourse.bass as bass
import concourse.mybir as mybir
import concourse.tile as tile
import concourse.bass_utils as bass_utils

F32 = mybir.dt.float32
F32R = mybir.dt.float32r
F16 = mybir.dt.float16
BF16 = mybir.dt.bfloat16
I32 = mybir.dt.int32
I16 = mybir.dt.int16
U32 = mybir.dt.uint32
AF = mybir.ActivationFunctionType
ALU = mybir.AluOpType

# model dims (hardcoded per problem spec)
DIM = 256
N = 1024
G = 4
HEADS = 8
DH = 32
NDS = 256          # downsampled kv positions
QS = 512           # queries per core
DPG = 64           # channels per group
OFF_K = 6
DS = 4             # downsample stride
OFF_SCALE = 4.0
NCORES = 8

DELTA = 2.0 / 1023.0
POS0 = -2.05
K2 = 1023.0 / 255.0
WIN = 1088         # CPB table window length (delta/2 grid: 2*511+1 -> 1088)
PMAX = 3200        # CPB windowed-table rows
TLEN = PMAX + WIN  # underlying table length (delta/2 spacing)
XROWS = 1059       # kv table rows (pairs), indexed by floor(ppix)+17

# A&S 7.1.26 erf coefficients (|err| <= 1.5e-7)
ERF_P = 0.3275911
ERF_A = [0.254829592, -0.284496736, 1.421413741, -1.453152027, 1.061405429]

# packed_a (f32r, [64, 516], conv-critical): wtaps 0:384, wqT 384:448,
#   wproj 448:449, bodw row (p0) 449:513.
# packed_b (f32r, [64, 384]): wkTs 0:64, wvT 64:128, woT 128:384.
# rowA2c/rowB2n (index affine rows) ship via the separate [1,512] "rows2"
# input (DVE TSP requires equal base partitions for its SB tensor inputs).
PKA_C = 516
PKB_C = 384

_CACHED = {}


def _patch_act_tables():
    """Restrict activation-table selection to the single set that covers all
    ACT functions used by this kernel, so exactly one table load is emitted."""
    import concourse.hw_specs as hw_specs

    if getattr(bacc, "_deform_act_patch", False):
        return
    orig = hw_specs.get_activation_tables

    keep = "exp_and_others"

    def patched(module_arch):
        tabs = orig(module_arch)
        keep_funcs = tabs[keep]
        out = {}
        for name, funcs in tabs.items():
            if name == keep:
                out[name] = funcs
            else:
                out[name] = funcs - keep_funcs
        return out

    bacc.get_activation_tables = patched
    bacc._deform_act_patch = True


def build_nc():
    _patch_act_tables()
    nc = bacc.Bacc("TRN2", target_bir_lowering=False, debug=False, num_devices=NCORES)

    din = {}

    def dt_in(name, shape, dtype=F32):
        din[name] = nc.dram_tensor(name, shape, dtype, kind="ExternalInput")
        return din[name]

    dt_in("xg", [DPG, N], F32R)
    dt_in("xq", [DPG, QS], F32R)
    dt_in("packed_a", [DPG, PKA_C], F32R)
    dt_in("packed_b", [DPG, PKB_C], F32R)
    dt_in("rows2", [1, 2 * NDS], F32)
    dt_in("cpb_tab", [PMAX, 2 * WIN], F16)
    dt_in("xt2", [XROWS, 2 * DPG], F32)
    idx_scr = nc.dram_tensor("idx_scr", [1, 2 * NDS], F32, kind="Internal")
    y_out = nc.dram_tensor("y", [DIM, QS], F16, kind="ExternalOutput")
    dbg = {}
    if DEBUG:
        for nm, shp in [("dbg_conv", [DPG, NDS]), ("dbg_gl", [DPG, NDS]),
                        ("dbg_r", [1, NDS]), ("dbg_T2", [1, 2 * NDS]),
                        ("dbg_P2", [1, 2 * NDS]), ("dbg_idx", [16, 32]),
                        ("dbg_kv", [DPG, NDS]), ("dbg_k", [DPG, NDS]),
                        ("dbg_bias00", [128, QS]), ("dbg_logit00", [128, QS]),
                        ("dbg_avn", [DPG, QS])]:
            dbg[nm] = nc.dram_tensor(nm, shp, F32, kind="ExternalOutput")

    qh_off = 1  # xgp column offset of x (left zero pad)

    with tile.TileContext(nc) as tc:
        with (
            tc.tile_pool(name="const", bufs=1) as cst,
            tc.tile_pool(name="work", bufs=2) as wk,
            tc.tile_pool(name="rows", bufs=1) as rw,
            tc.tile_pool(name="pers", bufs=1) as pe_pool,
        ):
            # ---------- t=0: idle-engine prep ----------
            xgp = cst.tile([DPG, N + 4], F32R, name="xgp", tag="xgp")
            nc.gpsimd.memset(xgp[:, 0:1].bitcast(F32), 0.0)
            nc.gpsimd.memset(xgp[:, 1 + N:N + 4].bitcast(F32), 0.0)
            idx16 = cst.tile([128, 32], I16, name="idx16", tag="idx16")
            # tiled identity [16, 128]: eye16[c, j] = (j % 16 == c), for
            # replicating the idx block to all 8 Q7 16-partition groups
            eyeio16 = cst.tile([16, 128], I32, name="eyeio16", tag="eyeio16")
            nc.gpsimd.iota(eyeio16[:], pattern=[[0, 8], [1, 16]], base=0,
                           channel_multiplier=-1)
            eye16 = cst.tile([16, 128], F32, name="eye16", tag="eye16")
            nc.vector.tensor_scalar(eye16[:], eyeio16[:], 0, None, ALU.is_equal)
            ones_row = cst.tile([1, NDS], F32R, name="ones_row", tag="ones_row")
            nc.gpsimd.memset(ones_row[:].bitcast(F32), 1.0)
            ones_col = cst.tile([128, 1], F32R, name="ones_col", tag="ones_col")
            nc.gpsimd.memset(ones_col[:].bitcast(F32), 1.0)
            # identity for PE transposes (f32)
            eyeio = cst.tile([128, 128], I32, name="eyeio", tag="eyeio")
            nc.gpsimd.iota(eyeio[:], pattern=[[1, 128]], base=0, channel_multiplier=-1)
            eyef = cst.tile([128, 128], F32, name="eyef", tag="eyef")
            nc.vector.tensor_scalar(eyef[:], eyeio[:], 0, None, ALU.is_equal)
            # warm the single ACT table at t=0 (overlaps input DMAs)
            wsrc = cst.tile([128, 1], F32, name="wsrc", tag="wsrc")
            nc.gpsimd.memset(wsrc[:], 0.0)
            warm = cst.tile([128, 1], F32, name="warm", tag="warm")
            nc.scalar.activation(warm[:], wsrc[:], AF.Relu)

            # ---------- input DMAs (packed on ACT queue; rest on SP) ----
            packed_a = cst.tile([DPG, PKA_C], F32R, name="packed_a", tag="packed_a")
            packed_b = cst.tile([DPG, PKB_C], F32R, name="packed_b", tag="packed_b")
            with tc.high_priority():
                nc.scalar.dma_start(packed_a[:], din["packed_a"].ap())
                nc.sync.dma_start(xgp[:, qh_off:qh_off + N], din["xg"].ap())
            nc.scalar.dma_start(packed_b[:], din["packed_b"].ap())
            xqt = cst.tile([DPG, QS], F32R, name="xqt", tag="xqt")
            nc.sync.dma_start(xqt[:], din["xq"].ap())
            rows2 = cst.tile([1, 2 * NDS], F32, name="rows2", tag="rows2")
            nc.sync.dma_start(rows2[:], din["rows2"].ap())
            wtaps = packed_a[0:DPG, 0:384]
            wqT = packed_a[0:DPG, 384:448]
            wproj = packed_a[0:DPG, 448:449]
            bodw_row = packed_a[0:1, 449:513]
            wkTs = packed_b[0:DPG, 0:64]
            wvT = packed_b[0:DPG, 64:128]
            woT = packed_b[0:DPG, 128:384]
            rowA2 = rows2[0:1, 0:NDS]
            rowB2 = rows2[0:1, NDS:2 * NDS]

            # persistent tiles crossing phases
            qs_sb = pe_pool.tile([DPG, QS], F32R, name="qs_sb", tag="qs_sb")
            k_sb = pe_pool.tile([DPG, NDS], F32R, name="k_sb", tag="k_sb")
            kv_sb = pe_pool.tile([DPG, NDS], F32R, name="kv_sb", tag="kv_sb")
            vT = [pe_pool.tile([128, DPG], F32R, name=f"vT{H}", tag=f"vT{H}")
                  for H in range(2)]
            fw = pe_pool.tile([128, 2], F32, name="fw", tag="fw")
            cpbg = pe_pool.tile([128, 2 * 2 * WIN], F16, name="cpbg", tag="cpbg")
            kvg = pe_pool.tile([128, 2 * 2 * DPG], F32, name="kvg", tag="kvg")
            avn = pe_pool.tile([DPG, QS], F32R, name="avn", tag="avn")

            with (
                tc.tile_pool(name="psA", bufs=1, space="PSUM") as psA,
                tc.tile_pool(name="psB", bufs=1, space="PSUM") as psB,
            ):
                # ---------- conv (strided depthwise fused with wq) ----------
                pconv = psA.tile([DPG, NDS], F32, name="pconv", tag="pconv")
                for k in range(OFF_K):
                    nc.tensor.matmul(
                        pconv[:], wtaps[:, 64 * k:64 * k + 64],
                        xgp[:, k:k + DS * (NDS - 1) + 1:DS],
                        start=(k == 0), stop=False)
                nc.tensor.matmul(pconv[:], bodw_row, ones_row[:],
                                 start=False, stop=True)
                if DEBUG:
                    dcv = wk.tile([DPG, NDS], F32, name="dcv", tag="dcv")
                    nc.vector.tensor_copy(dcv[:], pconv[:])
                    nc.sync.dma_start(dbg["dbg_conv"].ap(), dcv[:])

                # ---------- gelu (tanh approx, native ACT tanh) ----------
                # 2*gelu(x) = x * (1 + tanh(c1*(x + c2*x^3)))
                sq = wk.tile([DPG, NDS], F32, name="g_sq", tag="g_sq")
                nc.scalar.activation(sq[:], pconv[:], AF.Square)
                x3 = wk.tile([DPG, NDS], F32, name="g_x3", tag="g_x3")
                nc.vector.tensor_tensor(x3[:], sq[:], pconv[:], ALU.mult)
                arg = wk.tile([DPG, NDS], F32, name="g_arg", tag="g_arg")
                nc.vector.scalar_tensor_tensor(arg[:], x3[:], 0.044715, pconv[:],
                                               ALU.mult, ALU.add)
                tg = wk.tile([DPG, NDS], F32, name="g_tg", tag="g_tg")
                nc.scalar.activation(tg[:], arg[:], AF.Tanh,
                                     scale=0.7978845608028654)
                gl = wk.tile([DPG, NDS], F32R, name="g_gl", tag="g_gl")
                nc.vector.scalar_tensor_tensor(gl[:], tg[:], 1.0, pconv[:],
                                               ALU.add, ALU.mult)
                if DEBUG:
                    nc.sync.dma_start(dbg["dbg_gl"].ap(), gl[:].bitcast(F32))

                # ---------- proj + tanh (as r = 1/(e^{2p}+1)) ----------
                pproj = psA.tile([1, NDS], F32, name="pproj", tag="pproj")
                nc.tensor.matmul(pproj[:], wproj, gl[:])
                # qs early on PE (data ready; overlaps the row chain)
                pqs = psA.tile([DPG, QS], F32, name="pqs", tag="pqs")
                nc.tensor.matmul(pqs[:], wqT, xqt[:])
                nc.scalar.copy(qs_sb[:], pqs[:])

                th = rw.tile([1, NDS], F32, name="th", tag="th")
                nc.scalar.activation(th[:], pproj[:], AF.Tanh)
                if DEBUG:
                    nc.sync.dma_start(dbg["dbg_r"].ap(), th[:])

                # ---------- index row: u2 (CPB, delta/2 units, +.5 folded)
                # at [0:256), ppix+17 (kv) at [256:512)
                UX = rw.tile([1, 2 * NDS], F32, name="UX", tag="UX")
                nc.vector.scalar_tensor_tensor(
                    UX[0:1, 0:NDS], th[:], float(-8.0 * K2), rowA2,
                    ALU.mult, ALU.add)
                nc.vector.scalar_tensor_tensor(
                    UX[0:1, NDS:2 * NDS], th[:], float(4096.0 / 255.0), rowB2,
                    ALU.mult, ALU.add)
                UXI = rw.tile([1, 2 * NDS], I32, name="UXI", tag="UXI")
                nc.vector.tensor_copy(UXI[:], UX[:])
                UXC = rw.tile([1, 2 * NDS], F32, name="UXC", tag="UXC")
                nc.vector.tensor_copy(UXC[:], UXI[:])
                GT = rw.tile([1, 2 * NDS], F32, name="GT", tag="GT")
                nc.vector.tensor_tensor(GT[:], UXC[:], UX[:], ALU.is_gt)
                P2 = rw.tile([1, 2 * NDS], F32, name="P2", tag="P2")
                nc.vector.tensor_tensor(P2[:], UXC[:], GT[:], ALU.subtract)
                # fire the wrap round-trip ASAP (SP queue), THEN compute fracs
                nc.sync.dma_start(idx_scr.ap(), P2[:])
                idxw = rw.tile([16, 32], F32, name="idxw", tag="idxw")
                nc.sync.dma_start(
                    idxw[:], idx_scr.ap().rearrange("a (s p) -> (a p) s", p=16))
                F2 = rw.tile([1, 2 * NDS], F32, name="F2", tag="F2")
                nc.vector.tensor_tensor(F2[:], UX[:], P2[:], ALU.subtract)
                if DEBUG:
                    nc.sync.dma_start(dbg["dbg_T2"].ap(), UX[:])
                    nc.sync.dma_start(dbg["dbg_P2"].ap(), P2[:])

                # kv lerp weights to per-partition columns: fw = [w1_H0, w1_H1]
                ptf = psA.tile([128, 2], F32, name="ptf", tag="ptf")
                for H in range(2):
                    nc.tensor.transpose(ptf[:, H:H + 1],
                                        F2[0:1, NDS + 128 * H:NDS + 128 * (H + 1)],
                                        eyef[0:1, 0:1])
                nc.scalar.copy(fw[:], ptf[:])

                # replicate idx block to all 8 Q7 core groups via PE matmul
                pidx = psA.tile([128, 32], F32, name="pidx", tag="pidx")
                nc.tensor.matmul(pidx[:], eye16[:], idxw[:])
                nc.vector.tensor_copy(idx16[:], pidx[:])
                if DEBUG:
                    didx = wk.tile([16, 32], F32, name="didx", tag="didx")
                    nc.vector.tensor_copy(didx[:], idx16[0:16, :])
                    nc.sync.dma_start(dbg["dbg_idx"].ap(), didx[:])

                # ---------- gathers (SWDGE): kv first (unblocks k/v/psim) ----
                nc.gpsimd.dma_gather(
                    kvg[:].rearrange("p (b e) -> p b e", b=2),
                    din["xt2"].ap(), idx16[:, 16:32], NDS, NDS, 2 * DPG)
                nc.gpsimd.dma_gather(
                    cpbg[:].rearrange("p (b e) -> p b e", b=2),
                    din["cpb_tab"].ap(), idx16[:, 0:16], NDS, NDS, 2 * WIN)

                # ---------- kv lerp + transpose + k/v ----------
                kvT = wk.tile([128, 128], F32, name="kvT", tag="kvT")
                for H in range(2):
                    b = 2 * DPG * H
                    nc.vector.tensor_tensor(
                        kvT[:, 64 * H:64 * H + 64],
                        kvg[:, b + DPG:b + 2 * DPG], kvg[:, b:b + DPG],
                        ALU.subtract)
                    nc.vector.scalar_tensor_tensor(
                        kvT[:, 64 * H:64 * H + 64],
                        kvT[:, 64 * H:64 * H + 64], fw[:, H:H + 1],
                        kvg[:, b:b + DPG], ALU.mult, ALU.add)
                for H in range(2):
                    pkv = psB.tile([DPG, 128], F32, name="pkv", tag="pkv")
                    nc.tensor.transpose(pkv[:], kvT[:, 64 * H:64 * H + 64],
                                        eyef[:])
                    nc.scalar.copy(kv_sb[:, 128 * H:128 * (H + 1)], pkv[:])
                if DEBUG:
                    nc.sync.dma_start(dbg["dbg_kv"].ap(), kv_sb[:].bitcast(F32))

                pk = psA.tile([DPG, NDS], F32, name="pk", tag="pk")
                nc.tensor.matmul(pk[:], wkTs, kv_sb[:])
                nc.scalar.copy(k_sb[:], pk[:])
                if DEBUG:
                    nc.sync.dma_start(dbg["dbg_k"].ap(), k_sb[:].bitcast(F32))
                for H in range(2):
                    pvT = psB.tile([128, DPG], F32, name="pvT", tag="pvT")
                    nc.tensor.matmul(pvT[:], kv_sb[:, 128 * H:128 * (H + 1)], wvT)
                    nc.scalar.copy(vT[H][:], pvT[:])

            # ---------- attention ----------
            with (
                tc.tile_pool(name="psS", bufs=2, space="PSUM") as psS,
                tc.tile_pool(name="psY", bufs=1, space="PSUM") as psY,
                tc.tile_pool(name="psE", bufs=2, space="PSUM") as psE,
            ):
                psims = {}
                eps = {}
                for h in range(2):
                    for H in range(2):
                        ps = psS.tile([128, QS], F32, name="psim", tag="psim")
                        nc.tensor.matmul(
                            ps[:], k_sb[32 * h:32 * (h + 1), 128 * H:128 * (H + 1)],
                            qs_sb[32 * h:32 * (h + 1), :])
                        psims[(h, H)] = ps
                        # exp(sim) on ACT right away (PSUM -> SBUF)
                        ep = wk.tile([128, QS], F32, name="ep", tag="ep")
                        nc.scalar.activation(ep[:], ps[:], AF.Exp)
                        eps[(h, H)] = ep

                # table holds exp(G_o - C_o) on the delta/2 grid; nearest-
                # neighbor read (stride 2 along q): numer = exp(psim) * E
                ets = {}
                for h in range(2):
                    o = h
                    for H in range(2):
                        base = 2 * WIN * H + WIN * o
                        Rn = cpbg[:, base:base + 2 * QS:2]
                        if DEBUG and h == 0 and H == 0:
                            dbb = wk.tile([128, QS], F32, name="dbb", tag="dbb")
                            nc.vector.tensor_copy(dbb[:], Rn)
                            nc.sync.dma_start(dbg["dbg_bias00"].ap(), dbb[:])
                        et = wk.tile([128, QS], F32R, name=f"et{h}{H}",
                                     tag=f"et{h}{H}")
                        nc.vector.tensor_tensor(et[:], Rn, eps[(h, H)][:],
                                                ALU.mult)
                        ets[(h, H)] = et

                # softmax denominators + weighted values, interleaved across h
                psums, pavs = {}, {}
                for h in range(2):
                    psum_s = psE.tile([1, QS], F32, name="psum_s", tag="psum_s")
                    for H in range(2):
                        nc.tensor.matmul(psum_s[:], ones_col[:], ets[(h, H)][:],
                                         start=(H == 0), stop=(H == 1))
                    psums[h] = psum_s
                    pav = psE.tile([32, QS], F32, name="pav", tag="pav")
                    for H in range(2):
                        nc.tensor.matmul(pav[:], vT[H][:, 32 * h:32 * (h + 1)],
                                         ets[(h, H)][:],
                                         start=(H == 0), stop=(H == 1))
                    pavs[h] = pav
                for h in range(2):
                    rs = rw.tile([1, QS], F32, name="rs", tag=f"rs{h}")
                    nc.vector.reciprocal(rs[:], psums[h][:])
                    rsb = wk.tile([32, QS], F32, name="rsb", tag="rsb")
                    nc.gpsimd.partition_broadcast(rsb[:], rs[:])
                    nc.vector.tensor_tensor(avn[32 * h:32 * (h + 1), :],
                                            pavs[h][:], rsb[:], ALU.mult)
                if DEBUG:
                    nc.sync.dma_start(dbg["dbg_avn"].ap(), avn[:].bitcast(F32))

                # ---------- output projection (h-split accumulation so py
                # starts right after head 0's avn; fp16 output halves the DMA)
                pys = [psY.tile([128, QS], F32, name=f"py{m}", tag=f"py{m}")
                       for m in range(2)]
                for h in range(2):
                    for m in range(2):
                        nc.tensor.matmul(
                            pys[m][:],
                            woT[32 * h:32 * (h + 1), 128 * m:128 * (m + 1)],
                            avn[32 * h:32 * (h + 1), :],
                            start=(h == 0), stop=(h == 1))
                for m in range(2):
                    y_sb = wk.tile([128, QS], F16, name="y_sb", tag="y_sb")
                    nc.scalar.copy(y_sb[:], pys[m][:])
                    q = nc.sync if m == 0 else nc.scalar
                    q.dma_start(y_out.ap()[128 * m:128 * (m + 1), :], y_sb[:])

    nc.compile()
    return nc


def _build_cpb_table(w1, b1, w2, b2, w3):
    """Windowed fp16 table of exp(G_o(pos) - C_o) on the delta/2 grid (the
    per-o shift C_o cancels in softmax; exp-space lets the bias apply as a
    multiply after exp(sim)). Returns [PMAX, 2*WIN] fp16."""
    m = np.arange(TLEN, dtype=np.float64)
    pos = POS0 + m * (1.0 / 1023.0)
    t = np.sign(pos) * np.log1p(np.abs(pos))
    H1 = np.maximum(t[:, None] * w1[None, :] + b1[None, :], 0.0)
    H2 = np.maximum(H1 @ w2.T + b2[None, :], 0.0)
    B = H2 @ w3.T                                        # [TLEN, 2] (b3 dropped)
    E = np.exp(B - B.max(axis=0, keepdims=True))
    E = np.maximum(E, 6.2e-5)   # keep fp16 normal; only where attn weight ~0
    sw = np.lib.stride_tricks.sliding_window_view(E, WIN, axis=0)  # [TLEN-WIN+1, 2, WIN]
    return np.ascontiguousarray(
        sw[:PMAX].reshape(PMAX, 2 * WIN)).astype(np.float16)


def _shard_inputs(inputs):
    x = np.ascontiguousarray(inputs["x"][0]).astype(np.float32)   # [256, 1024]
    wq, wk, wv = inputs["wq"], inputs["wk"], inputs["wv"]
    wo = inputs["wo"]
    w_off_dw = inputs["w_off_dw"][:, 0, :]                 # [64, 6]
    b_off_dw = inputs["b_off_dw"]
    w_off_proj = inputs["w_off_proj"]
    w1 = np.asarray(inputs["cpb_w1"][:, 0], np.float64)
    b1 = np.asarray(inputs["cpb_b1"], np.float64)
    w2 = np.asarray(inputs["cpb_w2"], np.float64)
    b2 = np.asarray(inputs["cpb_b2"], np.float64)
    w3 = np.asarray(inputs["cpb_w3"], np.float64)

    f = np.float32
    cpb_tab = _build_cpb_table(w1, b1, w2, b2, w3)

    j = np.arange(NDS, dtype=np.float64)
    rowB2 = (1024.0 / 255.0) * j + 16.5

    in_maps = []
    for c in range(NCORES):
        g, qh = c // 2, c % 2
        xg = np.ascontiguousarray(x[64 * g:64 * (g + 1)], dtype=f)
        qbase = float(QS * qh)
        rowA2 = 2.0 * (qbase - POS0 * 511.5 - K2 * j) + 0.5

        pka = np.zeros((DPG, PKA_C), f)
        for k in range(OFF_K):
            pka[:, 64 * k:64 * k + 64] = wq[g].T * w_off_dw[None, :, k]
        pka[:, 384:448] = wq[g].T
        pka[:, 448] = 0.5 * w_off_proj
        pka[0, 449:513] = b_off_dw
        pkb = np.zeros((DPG, PKB_C), f)
        pkb[:, 0:64] = wk[g].T * f(DH) ** f(-0.5)
        pkb[:, 64:128] = wv[g].T
        pkb[:, 128:384] = wo[:, 64 * g:64 * (g + 1)].T

        xpad = np.zeros((XROWS + 1, DPG), f)
        xpad[17:17 + N] = xg.T
        xt2 = np.concatenate([xpad[:-1], xpad[1:]], axis=1)  # [1059, 128]

        rows2 = np.concatenate([rowA2, rowB2]).astype(f)[None, :]
        in_maps.append({
            "xg": xg,
            "rows2": rows2,
            "xq": np.ascontiguousarray(xg[:, QS * qh:QS * (qh + 1)]),
            "packed_a": pka,
            "packed_b": pkb,
            "cpb_tab": cpb_tab,
            "xt2": np.ascontiguousarray(xt2),
        })
    return in_maps


def kernel(**inputs):
    if "nc" not in _CACHED:
        _CACHED["nc"] = build_nc()
    nc = _CACHED["nc"]
    in_maps = _shard_inputs(inputs)
    res = bass_utils.run_bass_kernel_spmd(nc, in_maps, core_ids=list(range(NCORES)))
    ys = [res.results[c]["y"] for c in range(NCORES)]
    bo = inputs["bo"]
    out = np.zeros((1, DIM, N), np.float32)
    for qh in range(2):
        acc = np.zeros((DIM, QS), np.float64)
        for g in range(G):
            acc += ys[2 * g + qh]
        out[0, :, QS * qh:QS * (qh + 1)] = (
            acc + bo.astype(np.float64)[:, None]).astype(np.float32)
    return out


# revision 36
# speedup vs baseline: 4.4482x; 1.1558x over previous
"""DeformableAttention1D on 8 TRN2 NeuronCores via Bass/Tile.

Sharding: core c handles offset-group g=c//2 (64 of 256 channels, 2 of 8 heads)
and query-half qh=c%2 (512 of 1024 positions). Each core computes its group's
offsets/gather/CPB/attention independently; the final output projection is
computed as a partial (wo sliced by group) and summed on the host.

Key idea vs the one-hot/MLP baseline: both the grid_sample gather AND the CPB
relative-position-bias MLP are evaluated via SWDGE dma_gather from
host-precomputed DRAM tables.

  * kv gather: rows of x^T (zero-padded, pairs [x_i | x_{i+1}]) indexed by
    floor(pixel coord); bilinear lerp is 2 DVE ops with per-partition weights.
  * CPB bias: bias(q,j,o) = G_o(pos) with pos = grid_q[q] - vgs[j] and G_o a
    fixed scalar function of the CPB weights only. grid_q is a uniform grid
    with spacing delta = 2/1023, so for fixed j the 512 query positions read a
    CONTIGUOUS window of a delta-spaced table of G_o. One dma_gather of 256
    windowed rows (fp16) + a per-partition lerp replaces the whole MLP.
    (b3 is dropped: constant per (o,q) shift cancels in softmax.)

The ACT engine is restricted to ONE table set (exp_and_others: Exp, Tanh,
Square, Copy, Relu, ...); gelu uses the tanh approximation natively.
"""
import os
import sys

sys.path.insert(0, "/opt/trn_rl_repo")

DEBUG = bool(os.environ.get("DEFORM_DEBUG"))

import numpy as np

import concourse.bacc as bacc
import concourse.bass as bass
import concourse.mybir as mybir
import concourse.tile as tile
import concourse.bass_utils as bass_utils

F32 = mybir.dt.float32
F32R = mybir.dt.float32r
F16 = mybir.dt.float16
BF16 = mybir.dt.bfloat16
I32 = mybir.dt.int32
I16 = mybir.dt.int16
U32 = mybir.dt.uint32
AF = mybir.ActivationFunctionType
ALU = mybir.AluOpType

# model dims (hardcoded per problem spec)
DIM = 256
N = 1024
G = 4
HEADS = 8
DH = 32
NDS = 256          # downsampled kv positions
QS = 512           # queries per core
DPG = 64           # channels per group
OFF_K = 6
DS = 4             # downsample stride
OFF_SCALE = 4.0
NCORES = 8

DELTA = 2.0 / 1023.0
POS0 = -2.05
K2 = 1023.0 / 255.0
WIN = 1088         # CPB table window length (delta/2 grid: 2*511+1 -> 1088)
PMAX = 3200        # CPB windowed-table rows
TLEN = PMAX + WIN  # underlying table length (delta/2 spacing)
XROWS = 1059       # kv table rows (pairs), indexed by floor(ppix)+17

# A&S 7.1.26 erf coefficients (|err| <= 1.5e-7)
ERF_P = 0.3275911
ERF_A = [0.254829592, -0.284496736, 1.421413741, -1.453152027, 1.061405429]

# packed_a (f32r, [64, 516], conv-critical): wtaps 0:384, wqT 384:448,
#   wproj 448:449, bodw row (p0) 449:513.
# packed_b (f32r, [64, 384]): wkTs 0:64, wvT 64:128, woT 128:384.
# rowA2c/rowB2n (index affine rows) ship via the separate [1,512] "rows2"
# input (DVE TSP requires equal base partitions for its SB tensor inputs).
PKA_C = 516
PKB_C = 384

_CACHED = {}


def _patch_act_tables():
    """Restrict activation-table selection to the single set that covers all
    ACT functions used by this kernel, so exactly one table load is emitted."""
    import concourse.hw_specs as hw_specs

    if getattr(bacc, "_deform_act_patch", False):
        return
    orig = hw_specs.get_activation_tables

    keep = "exp_and_others"

    def patched(module_arch):
        tabs = orig(module_arch)
        keep_funcs = tabs[keep]
        out = {}
        for name, funcs in tabs.items():
            if name == keep:
                out[name] = funcs
            else:
                out[name] = funcs - keep_funcs
        return out

    bacc.get_activation_tables = patched
    bacc._deform_act_patch = True


def build_nc():
    _patch_act_tables()
    nc = bacc.Bacc("TRN2", target_bir_lowering=False, debug=False, num_devices=NCORES)

    din = {}

    def dt_in(name, shape, dtype=F32):
        din[name] = nc.dram_tensor(name, shape, dtype, kind="ExternalInput")
        return din[name]

    dt_in("xg", [DPG, N], F32R)
    dt_in("xq", [DPG, QS], F32R)
    dt_in("packed_a", [DPG, PKA_C], F32R)
    dt_in("packed_b", [DPG, PKB_C], F32R)
    dt_in("rows2", [1, 2 * NDS], F32)
    dt_in("cpb_tab", [PMAX, 2 * WIN], F16)
    dt_in("xt2", [XROWS, 2 * DPG], F32)
    idx_scr = nc.dram_tensor("idx_scr", [1, 2 * NDS], F32, kind="Internal")
    y_out = nc.dram_tensor("y", [DIM, QS], F16, kind="ExternalOutput")
    dbg = {}
    if DEBUG:
        for nm, shp in [("dbg_conv", [DPG, NDS]), ("dbg_gl", [DPG, NDS]),
                        ("dbg_r", [1, NDS]), ("dbg_T2", [1, 2 * NDS]),
                        ("dbg_P2", [1, 2 * NDS]), ("dbg_idx", [16, 32]),
                        ("dbg_kv", [DPG, NDS]), ("dbg_k", [DPG, NDS]),
                        ("dbg_bias00", [128, QS]), ("dbg_logit00", [128, QS]),
                        ("dbg_avn", [DPG, QS])]:
            dbg[nm] = nc.dram_tensor(nm, shp, F32, kind="ExternalOutput")

    qh_off = 1  # xgp column offset of x (left zero pad)

    with tile.TileContext(nc) as tc:
        with (
            tc.tile_pool(name="const", bufs=1) as cst,
            tc.tile_pool(name="work", bufs=2) as wk,
            tc.tile_pool(name="rows", bufs=1) as rw,
            tc.tile_pool(name="pers", bufs=1) as pe_pool,
        ):
            # ---------- t=0: idle-engine prep ----------
            xgp = cst.tile([DPG, N + 4], F32R, name="xgp", tag="xgp")
            nc.gpsimd.memset(xgp[:, 0:1].bitcast(F32), 0.0)
            nc.gpsimd.memset(xgp[:, 1 + N:N + 4].bitcast(F32), 0.0)
            idx16 = cst.tile([128, 32], I16, name="idx16", tag="idx16")
            # tiled identity [16, 128]: eye16[c, j] = (j % 16 == c), for
            # replicating the idx block to all 8 Q7 16-partition groups
            eyeio16 = cst.tile([16, 128], I32, name="eyeio16", tag="eyeio16")
            nc.gpsimd.iota(eyeio16[:], pattern=[[0, 8], [1, 16]], base=0,
                           channel_multiplier=-1)
            eye16 = cst.tile([16, 128], F32, name="eye16", tag="eye16")
            nc.vector.tensor_scalar(eye16[:], eyeio16[:], 0, None, ALU.is_equal)
            ones_row = cst.tile([1, NDS], F32R, name="ones_row", tag="ones_row")
            nc.gpsimd.memset(ones_row[:].bitcast(F32), 1.0)
            ones_col = cst.tile([128, 1], F32R, name="ones_col", tag="ones_col")
            nc.gpsimd.memset(ones_col[:].bitcast(F32), 1.0)
            # identity for PE transposes (f32)
            eyeio = cst.tile([128, 128], I32, name="eyeio", tag="eyeio")
            nc.gpsimd.iota(eyeio[:], pattern=[[1, 128]], base=0, channel_multiplier=-1)
            eyef = cst.tile([128, 128], F32, name="eyef", tag="eyef")
            nc.vector.tensor_scalar(eyef[:], eyeio[:], 0, None, ALU.is_equal)
            # warm the single ACT table at t=0 (overlaps input DMAs)
            wsrc = cst.tile([128, 1], F32, name="wsrc", tag="wsrc")
            nc.gpsimd.memset(wsrc[:], 0.0)
            warm = cst.tile([128, 1], F32, name="warm", tag="warm")
            nc.scalar.activation(warm[:], wsrc[:], AF.Relu)
            # PE p-state warmup fodder
            wmm = cst.tile([128, 128], F32R, name="wmm", tag="wmm")
            nc.gpsimd.memset(wmm[:].bitcast(F32), 0.0)

            # ---------- input DMAs (packed on ACT queue; rest on SP) ----
            packed_a = cst.tile([DPG, PKA_C], F32R, name="packed_a", tag="packed_a")
            packed_b = cst.tile([DPG, PKB_C], F32R, name="packed_b", tag="packed_b")
            with tc.high_priority():
                nc.scalar.dma_start(packed_a[:], din["packed_a"].ap())
                nc.sync.dma_start(xgp[:, qh_off:qh_off + N], din["xg"].ap())
            nc.scalar.dma_start(packed_b[:], din["packed_b"].ap())
            xqt = cst.tile([DPG, QS], F32R, name="xqt", tag="xqt")
            nc.sync.dma_start(xqt[:], din["xq"].ap())
            rows2 = cst.tile([1, 2 * NDS], F32, name="rows2", tag="rows2")
            nc.sync.dma_start(rows2[:], din["rows2"].ap())
            wtaps = packed_a[0:DPG, 0:384]
            wqT = packed_a[0:DPG, 384:448]
            wproj = packed_a[0:DPG, 448:449]
            bodw_row = packed_a[0:1, 449:513]
            wkTs = packed_b[0:DPG, 0:64]
            wvT = packed_b[0:DPG, 64:128]
            woT = packed_b[0:DPG, 128:384]
            rowA2 = rows2[0:1, 0:NDS]
            rowB2 = rows2[0:1, NDS:2 * NDS]

            # persistent tiles crossing phases
            qs_sb = pe_pool.tile([DPG, QS], F32R, name="qs_sb", tag="qs_sb")
            k_sb = pe_pool.tile([DPG, NDS], F32R, name="k_sb", tag="k_sb")
            kv_sb = pe_pool.tile([DPG, NDS], F32R, name="kv_sb", tag="kv_sb")
            vT = [pe_pool.tile([128, DPG], F32R, name=f"vT{H}", tag=f"vT{H}")
                  for H in range(2)]
            fw = pe_pool.tile([128, 2], F32, name="fw", tag="fw")
            cpbg = pe_pool.tile([128, 2 * 2 * WIN], F16, name="cpbg", tag="cpbg")
            kvg = pe_pool.tile([128, 2 * 2 * DPG], F32, name="kvg", tag="kvg")
            avn = pe_pool.tile([DPG, QS], F32R, name="avn", tag="avn")

            with (
                tc.tile_pool(name="psA", bufs=1, space="PSUM") as psA,
                tc.tile_pool(name="psB", bufs=1, space="PSUM") as psB,
            ):
                # ---------- conv (strided depthwise fused with wq) ----------
                pconv = psA.tile([DPG, NDS], F32, name="pconv", tag="pconv")
                # PE clock warmup: dependency-free matmuls keep the ramp model
                # at full speed by the time real matmuls arrive
                for w in range(16):
                    nc.tensor.matmul(pconv[0:DPG, 0:64], wmm[:, 0:DPG],
                                     wmm[:, 0:64], skip_group_check=True)
                for k in range(OFF_K):
                    nc.tensor.matmul(
                        pconv[:], wtaps[:, 64 * k:64 * k + 64],
                        xgp[:, k:k + DS * (NDS - 1) + 1:DS],
                        start=(k == 0), stop=False)
                nc.tensor.matmul(pconv[:], bodw_row, ones_row[:],
                                 start=False, stop=True)
                if DEBUG:
                    dcv = wk.tile([DPG, NDS], F32, name="dcv", tag="dcv")
                    nc.vector.tensor_copy(dcv[:], pconv[:])
                    nc.sync.dma_start(dbg["dbg_conv"].ap(), dcv[:])

                # ---------- gelu (tanh approx, native ACT tanh) ----------
                # 2*gelu(x) = x * (1 + tanh(c1*(x + c2*x^3)))
                sq = wk.tile([DPG, NDS], F32, name="g_sq", tag="g_sq")
                nc.scalar.activation(sq[:], pconv[:], AF.Square)
                x3 = wk.tile([DPG, NDS], F32, name="g_x3", tag="g_x3")
                nc.vector.tensor_tensor(x3[:], sq[:], pconv[:], ALU.mult)
                arg = wk.tile([DPG, NDS], F32, name="g_arg", tag="g_arg")
                nc.vector.scalar_tensor_tensor(arg[:], x3[:], 0.044715, pconv[:],
                                               ALU.mult, ALU.add)
                tg = wk.tile([DPG, NDS], F32, name="g_tg", tag="g_tg")
                nc.scalar.activation(tg[:], arg[:], AF.Tanh,
                                     scale=0.7978845608028654)
                gl = wk.tile([DPG, NDS], F32R, name="g_gl", tag="g_gl")
                nc.vector.scalar_tensor_tensor(gl[:], tg[:], 1.0, pconv[:],
                                               ALU.add, ALU.mult)
                if DEBUG:
                    nc.sync.dma_start(dbg["dbg_gl"].ap(), gl[:].bitcast(F32))

                # ---------- proj + tanh (as r = 1/(e^{2p}+1)) ----------
                pproj = psA.tile([1, NDS], F32, name="pproj", tag="pproj")
                nc.tensor.matmul(pproj[:], wproj, gl[:])
                # qs early on PE (data ready; overlaps the row chain)
                pqs = psA.tile([DPG, QS], F32, name="pqs", tag="pqs")
                nc.tensor.matmul(pqs[:], wqT, xqt[:])
                nc.scalar.copy(qs_sb[:], pqs[:])

                th = rw.tile([1, NDS], F32, name="th", tag="th")
                nc.scalar.activation(th[:], pproj[:], AF.Tanh)
                if DEBUG:
                    nc.sync.dma_start(dbg["dbg_r"].ap(), th[:])

                # ---------- index row: u2 (CPB, delta/2 units, +.5 folded)
                # at [0:256), ppix+17 (kv) at [256:512)
                UX = rw.tile([1, 2 * NDS], F32, name="UX", tag="UX")
                nc.vector.scalar_tensor_tensor(
                    UX[0:1, 0:NDS], th[:], float(-8.0 * K2), rowA2,
                    ALU.mult, ALU.add)
                nc.vector.scalar_tensor_tensor(
                    UX[0:1, NDS:2 * NDS], th[:], float(4096.0 / 255.0), rowB2,
                    ALU.mult, ALU.add)
                # fire the wrap round-trip on the RAW row ASAP (SP queue);
                # floors happen post-wrap on [16, 32] (cheap) and, for the kv
                # lerp fraction, on the ppix half row (overlapped with the
                # round-trip).
                nc.sync.dma_start(idx_scr.ap(), UX[:])
                idxw = rw.tile([16, 32], F32, name="idxw", tag="idxw")
                nc.sync.dma_start(
                    idxw[:], idx_scr.ap().rearrange("a (s p) -> (a p) s", p=16))
                UPX = UX[0:1, NDS:2 * NDS]
                XI = rw.tile([1, NDS], I32, name="XI", tag="XI")
                nc.vector.tensor_copy(XI[:], UPX)
                XC = rw.tile([1, NDS], F32, name="XC", tag="XC")
                nc.vector.tensor_copy(XC[:], XI[:])
                XG = rw.tile([1, NDS], F32, name="XG", tag="XG")
                nc.vector.tensor_tensor(XG[:], XC[:], UPX, ALU.is_gt)
                XP = rw.tile([1, NDS], F32, name="XP", tag="XP")
                nc.vector.tensor_tensor(XP[:], XC[:], XG[:], ALU.subtract)
                F2 = rw.tile([1, NDS], F32, name="F2", tag="F2")
                nc.vector.tensor_tensor(F2[:], UPX, XP[:], ALU.subtract)
                if DEBUG:
                    nc.sync.dma_start(dbg["dbg_T2"].ap(), UX[:])

                # kv lerp weights to per-partition columns: fw = [w1_H0, w1_H1]
                ptf = psA.tile([128, 2], F32, name="ptf", tag="ptf")
                for H in range(2):
                    nc.tensor.transpose(ptf[:, H:H + 1],
                                        F2[0:1, 128 * H:128 * (H + 1)],
                                        eyef[0:1, 0:1])
                nc.scalar.copy(fw[:], ptf[:])

                # post-wrap floors on [16, 32], then replicate to all 8 Q7
                # core groups via PE matmul
                WI = rw.tile([16, 32], I32, name="WI", tag="WI")
                nc.vector.tensor_copy(WI[:], idxw[:])
                WC = rw.tile([16, 32], F32, name="WC", tag="WC")
                nc.vector.tensor_copy(WC[:], WI[:])
                WG = rw.tile([16, 32], F32, name="WG", tag="WG")
                nc.vector.tensor_tensor(WG[:], WC[:], idxw[:], ALU.is_gt)
                WP = rw.tile([16, 32], F32, name="WP", tag="WP")
                nc.vector.tensor_tensor(WP[:], WC[:], WG[:], ALU.subtract)
                pidx = psA.tile([128, 32], F32, name="pidx", tag="pidx")
                nc.tensor.matmul(pidx[:], eye16[:], WP[:])
                nc.vector.tensor_copy(idx16[:], pidx[:])
                if DEBUG:
                    didx = wk.tile([16, 32], F32, name="didx", tag="didx")
                    nc.vector.tensor_copy(didx[:], idx16[0:16, :])
                    nc.sync.dma_start(dbg["dbg_idx"].ap(), didx[:])

                # ---------- gathers (SWDGE): kv first (unblocks k/v/psim) ----
                nc.gpsimd.dma_gather(
                    kvg[:].rearrange("p (b e) -> p b e", b=2),
                    din["xt2"].ap(), idx16[:, 16:32], NDS, NDS, 2 * DPG)
                for Hs in range(2):
                    nc.gpsimd.dma_gather(
                        cpbg[:, 2 * WIN * Hs:2 * WIN * (Hs + 1)].rearrange(
                            "p (b e) -> p b e", b=1),
                        din["cpb_tab"].ap(), idx16[:, 8 * Hs:8 * Hs + 8],
                        128, 128, 2 * WIN)

                # ---------- kv lerp + transpose + k/v ----------
                kvT = wk.tile([128, 128], F32, name="kvT", tag="kvT")
                for H in range(2):
                    b = 2 * DPG * H
                    nc.vector.tensor_tensor(
                        kvT[:, 64 * H:64 * H + 64],
                        kvg[:, b + DPG:b + 2 * DPG], kvg[:, b:b + DPG],
                        ALU.subtract)
                    nc.vector.scalar_tensor_tensor(
                        kvT[:, 64 * H:64 * H + 64],
                        kvT[:, 64 * H:64 * H + 64], fw[:, H:H + 1],
                        kvg[:, b:b + DPG], ALU.mult, ALU.add)
                for H in range(2):
                    pkv = psB.tile([DPG, 128], F32, name="pkv", tag="pkv")
                    nc.tensor.transpose(pkv[:], kvT[:, 64 * H:64 * H + 64],
                                        eyef[:])
                    nc.scalar.copy(kv_sb[:, 128 * H:128 * (H + 1)], pkv[:])
                if DEBUG:
                    nc.sync.dma_start(dbg["dbg_kv"].ap(), kv_sb[:].bitcast(F32))

                pk = psA.tile([DPG, NDS], F32, name="pk", tag="pk")
                nc.tensor.matmul(pk[:], wkTs, kv_sb[:])
                nc.scalar.copy(k_sb[:], pk[:])
                if DEBUG:
                    nc.sync.dma_start(dbg["dbg_k"].ap(), k_sb[:].bitcast(F32))
                for H in range(2):
                    pvT = psB.tile([128, DPG], F32, name="pvT", tag="pvT")
                    nc.tensor.matmul(pvT[:], kv_sb[:, 128 * H:128 * (H + 1)], wvT)
                    nc.scalar.copy(vT[H][:], pvT[:])

            # ---------- attention ----------
            with (
                tc.tile_pool(name="psS", bufs=2, space="PSUM") as psS,
                tc.tile_pool(name="psY", bufs=1, space="PSUM") as psY,
                tc.tile_pool(name="psE", bufs=2, space="PSUM") as psE,
            ):
                psims = {}
                eps = {}
                for H in range(2):
                    for h in range(2):
                        ps = psS.tile([128, QS], F32, name="psim", tag="psim")
                        nc.tensor.matmul(
                            ps[:], k_sb[32 * h:32 * (h + 1), 128 * H:128 * (H + 1)],
                            qs_sb[32 * h:32 * (h + 1), :])
                        psims[(h, H)] = ps
                        # exp(sim) on ACT right away (PSUM -> SBUF)
                        ep = wk.tile([128, QS], F32, name="ep", tag="ep")
                        nc.scalar.activation(ep[:], ps[:], AF.Exp)
                        eps[(h, H)] = ep

                # table holds exp(G_o - C_o) on the delta/2 grid; nearest-
                # neighbor read (stride 2 along q): numer = exp(psim) * E
                ets = {}
                for H in range(2):
                    for h in range(2):
                        o = h
                        base = 2 * WIN * H + WIN * o
                        Rn = cpbg[:, base:base + 2 * QS:2]
                        if DEBUG and h == 0 and H == 0:
                            dbb = wk.tile([128, QS], F32, name="dbb", tag="dbb")
                            nc.vector.tensor_copy(dbb[:], Rn)
                            nc.sync.dma_start(dbg["dbg_bias00"].ap(), dbb[:])
                        et = wk.tile([128, QS], F32R, name=f"et{h}{H}",
                                     tag=f"et{h}{H}")
                        nc.vector.tensor_tensor(et[:], Rn, eps[(h, H)][:],
                                                ALU.mult)
                        ets[(h, H)] = et

                # softmax denominators + weighted values, interleaved across h
                psums, pavs = {}, {}
                for h in range(2):
                    psum_s = psE.tile([1, QS], F32, name="psum_s", tag="psum_s")
                    for H in range(2):
                        nc.tensor.matmul(psum_s[:], ones_col[:], ets[(h, H)][:],
                                         start=(H == 0), stop=(H == 1))
                    psums[h] = psum_s
                    pav = psE.tile([32, QS], F32, name="pav", tag="pav")
                    for H in range(2):
                        nc.tensor.matmul(pav[:], vT[H][:, 32 * h:32 * (h + 1)],
                                         ets[(h, H)][:],
                                         start=(H == 0), stop=(H == 1))
                    pavs[h] = pav
                for h in range(2):
                    rs = rw.tile([1, QS], F32, name="rs", tag=f"rs{h}")
                    nc.vector.reciprocal(rs[:], psums[h][:])
                    rsb = wk.tile([32, QS], F32, name="rsb", tag="rsb")
                    nc.gpsimd.partition_broadcast(rsb[:], rs[:])
                    nc.vector.tensor_tensor(avn[32 * h:32 * (h + 1), :],
                                            pavs[h][:], rsb[:], ALU.mult)
                if DEBUG:
                    nc.sync.dma_start(dbg["dbg_avn"].ap(), avn[:].bitcast(F32))

                # ---------- output projection (h-split accumulation so py
                # starts right after head 0's avn; fp16 output halves the DMA)
                pys = [psY.tile([128, QS], F32, name=f"py{m}", tag=f"py{m}")
                       for m in range(2)]
                for h in range(2):
                    for m in range(2):
                        nc.tensor.matmul(
                            pys[m][:],
                            woT[32 * h:32 * (h + 1), 128 * m:128 * (m + 1)],
                            avn[32 * h:32 * (h + 1), :],
                            start=(h == 0), stop=(h == 1))
                for m in range(2):
                    y_sb = wk.tile([128, QS], F16, name="y_sb", tag="y_sb")
                    nc.scalar.copy(y_sb[:], pys[m][:])
                    q = nc.sync if m == 0 else nc.scalar
                    q.dma_start(y_out.ap()[128 * m:128 * (m + 1), :], y_sb[:])

    nc.compile()
    return nc


def _build_cpb_table(w1, b1, w2, b2, w3):
    """Windowed fp16 table of exp(G_o(pos) - C_o) on the delta/2 grid (the
    per-o shift C_o cancels in softmax; exp-space lets the bias apply as a
    multiply after exp(sim)). Returns [PMAX, 2*WIN] fp16."""
    m = np.arange(TLEN, dtype=np.float64)
    pos = POS0 + m * (1.0 / 1023.0)
    t = np.sign(pos) * np.log1p(np.abs(pos))
    H1 = np.maximum(t[:, None] * w1[None, :] + b1[None, :], 0.0)
    H2 = np.maximum(H1 @ w2.T + b2[None, :], 0.0)
    B = H2 @ w3.T                                        # [TLEN, 2] (b3 dropped)
    E = np.exp(B - B.max(axis=0, keepdims=True))
    E = np.maximum(E, 6.2e-5)   # keep fp16 normal; only where attn weight ~0
    sw = np.lib.stride_tricks.sliding_window_view(E, WIN, axis=0)  # [TLEN-WIN+1, 2, WIN]
    return np.ascontiguousarray(
        sw[:PMAX].reshape(PMAX, 2 * WIN)).astype(np.float16)


def _shard_inputs(inputs):
    x = np.ascontiguousarray(inputs["x"][0]).astype(np.float32)   # [256, 1024]
    wq, wk, wv = inputs["wq"], inputs["wk"], inputs["wv"]
    wo = inputs["wo"]
    w_off_dw = inputs["w_off_dw"][:, 0, :]                 # [64, 6]
    b_off_dw = inputs["b_off_dw"]
    w_off_proj = inputs["w_off_proj"]
    w1 = np.asarray(inputs["cpb_w1"][:, 0], np.float64)
    b1 = np.asarray(inputs["cpb_b1"], np.float64)
    w2 = np.asarray(inputs["cpb_w2"], np.float64)
    b2 = np.asarray(inputs["cpb_b2"], np.float64)
    w3 = np.asarray(inputs["cpb_w3"], np.float64)

    f = np.float32
    cpb_tab = _build_cpb_table(w1, b1, w2, b2, w3)

    j = np.arange(NDS, dtype=np.float64)
    rowB2 = (1024.0 / 255.0) * j + 16.5

    in_maps = []
    for c in range(NCORES):
        g, qh = c // 2, c % 2
        xg = np.ascontiguousarray(x[64 * g:64 * (g + 1)], dtype=f)
        qbase = float(QS * qh)
        rowA2 = 2.0 * (qbase - POS0 * 511.5 - K2 * j) + 0.5

        pka = np.zeros((DPG, PKA_C), f)
        for k in range(OFF_K):
            pka[:, 64 * k:64 * k + 64] = wq[g].T * w_off_dw[None, :, k]
        pka[:, 384:448] = wq[g].T
        pka[:, 448] = 0.5 * w_off_proj
        pka[0, 449:513] = b_off_dw
        pkb = np.zeros((DPG, PKB_C), f)
        pkb[:, 0:64] = wk[g].T * f(DH) ** f(-0.5)
        pkb[:, 64:128] = wv[g].T
        pkb[:, 128:384] = wo[:, 64 * g:64 * (g + 1)].T

        xpad = np.zeros((XROWS + 1, DPG), f)
        xpad[17:17 + N] = xg.T
        xt2 = np.concatenate([xpad[:-1], xpad[1:]], axis=1)  # [1059, 128]

        rows2 = np.concatenate([rowA2, rowB2]).astype(f)[None, :]
        in_maps.append({
            "xg": xg,
            "rows2": rows2,
            "xq": np.ascontiguousarray(xg[:, QS * qh:QS * (qh + 1)]),
            "packed_a": pka,
            "packed_b": pkb,
            "cpb_tab": cpb_tab,
            "xt2": np.ascontiguousarray(xt2),
        })
    return in_maps


def kernel(**inputs):
    if "nc" not in _CACHED:
        _CACHED["nc"] = build_nc()
    nc = _CACHED["nc"]
    in_maps = _shard_inputs(inputs)
    res = bass_utils.run_bass_kernel_spmd(nc, in_maps, core_ids=list(range(NCORES)))
    ys = [res.results[c]["y"] for c in range(NCORES)]
    bo = inputs["bo"]
    out = np.zeros((1, DIM, N), np.float32)
    for qh in range(2):
        acc = np.zeros((DIM, QS), np.float64)
        for g in range(G):
            acc += ys[2 * g + qh]
        out[0, :, QS * qh:QS * (qh + 1)] = (
            acc + bo.astype(np.float64)[:, None]).astype(np.float32)
    return out
